# revision 46
# baseline (speedup 1.0000x reference)
"""Trainium2 Bass kernel for nn_AttentionModel (seq2seq LSTM with attention).

Sharding: pure data parallelism over batch (256 -> 8 cores x 32), all
weights replicated. Per-core layout keeps the hidden/gate dimension on
SBUF partitions and (time, batch) on the free axis so the recurrent
matmuls, elementwise gate math, and attention all use one consistent
layout with no on-device transposes.

v2 performance structure (the kernel is LDWEIGHTS-bound: per 128x128
weight tile the PE pays ~128 rows of load for only B=32 moving columns):
- all gate weights (whh_e/p/d, wih_e/p/d, linT) stored fp8 e4m3 and
  consumed with DoubleRow matmuls: each instruction contracts a 256-row
  k-pair, halving both weight-load time and instruction count
- decoder wih+whh gates accumulate into ONE psum tile (no identity-
  matmul merge passes, no psum->sbuf->psum round trips)
- encoder per-step gx add is 2 wide identity matmuls (psum init), not
  32 narrow ones
- the two encoder chains are interleaved so one chain's gate/cell
  elementwise tail (ACT/DVE) hides under the other chain's recurrent
  matmuls (PE)
- gates packed [i, f, o, g] so one Sigmoid covers i,f,o contiguously
- attention scores via a diagonal matmul (c stationary) so the softmax
  lands batch-on-partitions and runs as per-partition-scalar ops

Self-contained: includes the TileContext wait-split workaround and all
host-side packing. The graded entry point is kernel(**inputs).
"""

import numpy as np
import ml_dtypes

import concourse.bass as bass
import concourse.mybir as mybir
import concourse.tile as tile
from concourse.bass_isa import ReduceOp
from concourse.bass_utils import run_bass_kernel_spmd

BF16 = ml_dtypes.bfloat16
FP8 = ml_dtypes.float8_e4m3
FP32 = mybir.dt.float32
BF = mybir.dt.bfloat16
F8 = mybir.dt.float8e4

GATE_FP8 = True     # fp8 gate weights (2x faster LDWEIGHTS + DoubleRow)
DOUBLE_ROW = True   # contract k-tile pairs per matmul instruction

N_CORES = 8
B = 32            # batch per core
T_IN = 10
T_OUT = 25
H = 1024
F = 512
P = 66
G = 4 * H         # 4096 gates
KT = H // 128     # 8  k-tiles over hidden
FT = F // 128     # 4  k-tiles over feature
MT = G // 128     # 32 m-tiles over gates
TCAT = 2 * T_IN + 1   # 21 attention slots
SLOT_DEC = T_IN       # decoder h lives at slot 10

_MAX_WAITS = 1


def _apply_tile_wait_patches():
    """The walrus CoreV3 codegen in this container rejects instructions
    carrying more than one sync-wait command ("Too many sync wait
    commands"). Keep every instruction at <=1 wait by moving excess waits
    onto same-engine nops emitted immediately before the instruction."""
    import bass_rust
    from concourse.vector_clock import ScopedClock

    SyncInfo = bass_rust.SyncInfo

    def _split_waits(nc, inst):
        si = getattr(inst, "sync_info", None)
        if si is None or not si.on_wait or len(si.on_wait) <= _MAX_WAITS:
            return
        if inst.engine == mybir.EngineType.Unassigned:
            return
        waits = list(si.on_wait)
        si.on_wait = waits[:_MAX_WAITS]
        rest = waits[_MAX_WAITS:]
        eng = nc.engines[inst.engine]
        for i in range(0, len(rest), _MAX_WAITS):
            nop = eng.nop(nofuse=True, hint="wait_split")
            nop.ins.sync_info = SyncInfo(
                on_wait=rest[i:i + _MAX_WAITS], on_update=[]
            )

    orig_commit = tile.TileContext._commit_instruction

    def _commit_split(self, inst, lazy_reg_writes=True):
        si = getattr(inst, "sync_info", None)
        if (si is not None and si.on_wait is not None
                and len(si.on_wait) > _MAX_WAITS
                and inst.engine != mybir.EngineType.Unassigned):
            _split_waits(self.nc, inst)
        return orig_commit(self, inst, lazy_reg_writes)

    tile.TileContext._commit_instruction = _commit_split

    def _drain_and_barrier_split(self, tick_clock, wait_clock):
        drain_inst = self.nc.sync.drain()
        wait_clock.add_sem_waits(
            drain_inst.ins, ScopedClock({None: tick_clock.global_clock})
        )
        sync_info = drain_inst.ins.sync_info
        if sync_info is not None and sync_info.on_wait is not None:
            waits = list(sync_info.on_wait)
            if len(waits) > _MAX_WAITS:
                sync_info.on_wait = waits[:_MAX_WAITS]
                rest = waits[_MAX_WAITS:]
                for i in range(0, len(rest), _MAX_WAITS):
                    nop = self.nc.sync.nop(nofuse=True, hint="drain_wait_split")
                    nop.ins.sync_info = SyncInfo(
                        on_wait=rest[i:i + _MAX_WAITS], on_update=[]
                    )
        self.nc.all_engine_barrier()
        assert self.sems is not None
        popped = self.nc._tile_sem_poison_stack.pop()
        assert popped is self._sem_poison
        self.nc.clear_and_free_semaphores(list(self.sems.allocated().values()))
        self.nc.all_engine_barrier()

    tile.TileContext._drain_and_barrier = _drain_and_barrier_split


_apply_tile_wait_patches()

# Opt-in: flip walrus --enable-ldw-opt (hardcoded false in
# bass_utils.bir_verify_and_optimise) by rewriting the command line.
LDW_OPT = False


def _apply_ldw_opt_patch():
    from concourse import bass_utils as _bu
    if getattr(_bu, "_ldw_opt_patched", False):
        return
    _bu._ldw_opt_patched = True
    _orig_run_command = _bu.run_command

    def _run_command_ldw(argv, **kwargs):
        if LDW_OPT:
            argv = ["--enable-ldw-opt=true" if a == "--enable-ldw-opt=false"
                    else a for a in argv]
        return _orig_run_command(argv, **kwargs)

    _bu.run_command = _run_command_ldw


_apply_ldw_opt_patch()


# ------------------------------------------------------------- host packing

# gate reorder: reference packs gates [i, f, g, o]; we use [i, f, o, g]
# so the three sigmoids (i, f, o) are contiguous.
_GPERM = np.concatenate([
    np.arange(0, H),              # i
    np.arange(H, 2 * H),          # f
    np.arange(3 * H, 4 * H),      # o
    np.arange(2 * H, 3 * H),      # g
])


def _pack_T(w, ktiles, mcols, dt=BF16):
    """(mcols, ktiles*128) weight -> transposed tiled layout
    (128, ktiles*mcols) with [p, kt*mcols + m] = w[m, kt*128 + p]."""
    wT = np.ascontiguousarray(np.asarray(w, np.float32).T).astype(dt)
    return np.ascontiguousarray(
        wT.reshape(ktiles, 128, mcols).transpose(1, 0, 2)
        .reshape(128, ktiles * mcols))


WS = 16.0   # fp8 pre-quantization scale: lifts glorot weights and unit
            # activations out of e4m3's subnormal range (the PE flushes
            # subnormal fp8 operands); gates come out x256, undone by the
            # activation's scale argument.


def _prep_weights(inputs, gate_fp8=GATE_FP8):
    gdt = FP8 if gate_fp8 else BF16
    ws = WS if gate_fp8 else 1.0
    d = {}
    d["tfT"] = np.ascontiguousarray(
        np.asarray(inputs["tf_w"], np.float32).T).astype(BF16)  # (66, 512)
    for nm, wih, whh in (("e", "enc_wih", "enc_whh"),
                         ("p", "encp_wih", "encp_whh"),
                         ("d", "dec_wih", "dec_whh")):
        d[f"wih_{nm}"] = _pack_T(
            np.asarray(inputs[wih], np.float32)[_GPERM] * ws, FT, G, gdt)
        d[f"whh_{nm}"] = _pack_T(
            np.asarray(inputs[whh], np.float32)[_GPERM] * ws, KT, G, gdt)
    d["linT"] = _pack_T(
        np.asarray(inputs["lin_w"], np.float32) * ws, KT, F, gdt)
    d["tpT"] = _pack_T(inputs["tp_w"], KT, P)            # (128, 8*66)
    d["b_tf"] = np.asarray(inputs["tf_b"], np.float32)
    for nm, bi, bh in (("e", "enc_bih", "enc_bhh"),
                       ("p", "encp_bih", "encp_bhh"),
                       ("d", "dec_bih", "dec_bhh")):
        d[f"b_{nm}"] = (np.asarray(inputs[bi], np.float32)
                        + np.asarray(inputs[bh], np.float32))[_GPERM] * ws * ws
    d["b_lin"] = np.asarray(inputs["lin_b"], np.float32) * ws * ws
    d["b_tp"] = np.asarray(inputs["tp_b"], np.float32)
    return d


def _bias_flags(w):
    return tuple(bool(np.any(w[k])) for k in
                 ("b_tf", "b_e", "b_p", "b_d", "b_lin", "b_tp"))


# ------------------------------------------------------------ device build

def build_model(bias_flags=(False,) * 6, loop_iters=1, gate_fp8=GATE_FP8,
                double_row=DOUBLE_ROW, ablate=(), warm_fillers=True):
    has_btf, has_be, has_bp, has_bd, has_blin, has_btp = bias_flags
    any_bias = any(bias_flags)
    GDT = F8 if gate_fp8 else BF
    PM = (mybir.MatmulPerfMode.DoubleRow
          if (gate_fp8 and double_row) else None)

    ws = WS if gate_fp8 else 1.0
    gsc = 1.0 / (ws * ws)   # gates psum carries ws^2; undone inside ACT

    nc = bass.Bass()

    xT_d = nc.dram_tensor("xT", [P, T_IN * B], BF, kind="ExternalInput")
    zT_d = nc.dram_tensor("zT", [P, T_IN * B], BF, kind="ExternalInput")
    residT_d = nc.dram_tensor("residT", [P, T_OUT * B], FP32,
                              kind="ExternalInput")
    tfT_d = nc.dram_tensor("tfT", [P, F], BF, kind="ExternalInput")
    wih_d_d = {}
    whh_d_d = {}
    for nm in ("e", "p", "d"):
        wih_d_d[nm] = nc.dram_tensor(f"wih_{nm}", [128, FT * G], GDT,
                                     kind="ExternalInput")
        whh_d_d[nm] = nc.dram_tensor(f"whh_{nm}", [128, KT * G], GDT,
                                     kind="ExternalInput")
    linT_d = nc.dram_tensor("linT", [128, KT * F], GDT, kind="ExternalInput")
    dmask_d = nc.dram_tensor("dmask", [B, B * TCAT], BF, kind="ExternalInput")
    tpT_d = nc.dram_tensor("tpT", [128, KT * P], BF, kind="ExternalInput")
    bias_d = {}
    if has_btf:
        bias_d["b_tf"] = nc.dram_tensor("b_tf", [1, F], BF,
                                        kind="ExternalInput")
    if has_be:
        bias_d["b_e"] = nc.dram_tensor("b_e", [1, G], BF,
                                       kind="ExternalInput")
    if has_bp:
        bias_d["b_p"] = nc.dram_tensor("b_p", [1, G], BF,
                                       kind="ExternalInput")
    if has_bd:
        bias_d["b_d"] = nc.dram_tensor("b_d", [1, G], BF,
                                       kind="ExternalInput")
    if has_blin:
        bias_d["b_lin"] = nc.dram_tensor("b_lin", [1, F], BF,
                                         kind="ExternalInput")
    if has_btp:
        bias_d["b_tp"] = nc.dram_tensor("b_tp", [1, P], BF,
                                        kind="ExternalInput")
    out_d = nc.dram_tensor("oT", [P, T_OUT * B], FP32, kind="ExternalOutput")

    with tile.TileContext(nc) as tc:
        with (
            tc.tile_pool(name="singles", bufs=1) as singles,
            tc.tile_pool(name="wih_pool", bufs=3) as wih_pool,
            tc.tile_pool(name="whh_pool", bufs=2) as whh_pool,
            tc.tile_pool(name="gx_pool", bufs=2) as gx_pool,
            tc.tile_pool(name="ew_pool", bufs=3) as ew_pool,
            tc.tile_pool(name="pA_pool", bufs=1, space="PSUM") as pA_pool,
            tc.tile_pool(name="pB_pool", bufs=1, space="PSUM") as pB_pool,
            tc.tile_pool(name="pC_pool", bufs=1, space="PSUM") as pC_pool,
            tc.tile_pool(name="mps_pool", bufs=2, space="PSUM") as mps_pool,
        ):
            def body(_it=None):
                def cast_scaled(out, in_, s):
                    # fp8 cast with pre-scale (lifts values out of the
                    # subnormal-flush range of the PE's fp8 datapath)
                    if s == 1.0:
                        nc.vector.tensor_copy(out=out, in_=in_)
                    else:
                        nc.vector.tensor_scalar_mul(out, in_, float(s))

                # k_loop: contract kt-tiles of w3 [128, ktn, M] against
                # mov3 [128, ktn, N] into psum `out`; DoubleRow pairs
                # k-tiles when enabled.
                def k_loop(out, w3, mov3, ktn, first, last, pm=None):
                    # pm=DoubleRow only pays off for wide moving operands:
                    # a DR stationary loads at 1 row/cycle unoverlapped
                    # (256 cyc), while consecutive non-DR matmuls hide the
                    # weight load under the previous matmul's moving phase
                    # (~64-74 cyc for K=128). Narrow (B=32) k-loops must
                    # stay non-DR.
                    if pm is not None:
                        for kp in range(ktn // 2):
                            nc.tensor.matmul(
                                out, w3[:, 2 * kp:2 * kp + 2, :],
                                mov3[:, 2 * kp:2 * kp + 2, :],
                                start=(first and kp == 0),
                                stop=(last and kp == ktn // 2 - 1),
                                perf_mode=pm)
                    else:
                        for kt in range(ktn):
                            nc.tensor.matmul(
                                out, w3[:, kt, :], mov3[:, kt, :],
                                start=(first and kt == 0),
                                stop=(last and kt == ktn - 1))

                # ------------- constant/static loads --------------------
                tfT = singles.tile([P, F], BF, tag="tfT")
                nc.sync.dma_start(out=tfT, in_=tfT_d[:, :])
                xT = singles.tile([P, T_IN * B], BF, tag="xT")
                nc.sync.dma_start(out=xT, in_=xT_d[:, :])
                zT = singles.tile([P, T_IN * B], BF, tag="zT")
                nc.sync.dma_start(out=zT, in_=zT_d[:, :])

                wih_e = wih_pool.tile([128, FT * G], GDT, tag="wih")
                for kt in range(FT):
                    nc.sync.dma_start(out=wih_e[:, kt * G:(kt + 1) * G],
                                      in_=wih_d_d["e"][:, kt * G:(kt + 1) * G])
                wih_p = wih_pool.tile([128, FT * G], GDT, tag="wih")
                for kt in range(FT):
                    nc.sync.dma_start(out=wih_p[:, kt * G:(kt + 1) * G],
                                      in_=wih_d_d["p"][:, kt * G:(kt + 1) * G])
                whh_e = whh_pool.tile([128, KT * G], GDT, tag="whh")
                for kt in range(KT):
                    nc.sync.dma_start(out=whh_e[:, kt * G:(kt + 1) * G],
                                      in_=whh_d_d["e"][:, kt * G:(kt + 1) * G])
                whh_p = whh_pool.tile([128, KT * G], GDT, tag="whh")
                for kt in range(KT):
                    nc.sync.dma_start(out=whh_p[:, kt * G:(kt + 1) * G],
                                      in_=whh_d_d["p"][:, kt * G:(kt + 1) * G])

                residT = singles.tile([P, T_OUT * B], FP32, tag="residT")
                nc.sync.dma_start(out=residT, in_=residT_d[:, :])
                linT = singles.tile([128, KT * F], GDT, tag="linT")
                for kt in range(KT):
                    nc.sync.dma_start(out=linT[:, kt * F:(kt + 1) * F],
                                      in_=linT_d[:, kt * F:(kt + 1) * F])
                tpT = singles.tile([128, KT * P], BF, tag="tpT")
                nc.sync.dma_start(out=tpT, in_=tpT_d[:, :])
                dmask = singles.tile([B, B * TCAT], BF, tag="dmask")
                nc.sync.dma_start(out=dmask, in_=dmask_d[:, :])

                bias_sb = {}
                for key, dram in bias_d.items():
                    t = singles.tile(list(dram.shape), BF, tag=key)
                    nc.sync.dma_start(out=t, in_=dram[:, :])
                    bias_sb[key] = t

                ones32 = singles.tile([B, 128], BF, tag="ones32")
                nc.vector.memset(ones32, 1.0)
                ident = singles.tile([128, 128], BF, tag="ident")
                from concourse.masks import make_identity
                make_identity(nc, ident)
                if any_bias:
                    ones_n = singles.tile([1, T_IN * B], BF, tag="ones_n")
                    nc.vector.memset(ones_n, 1.0)

                cat = singles.tile([128, KT, B, TCAT], BF, tag="cat")
                c_e = singles.tile([128, KT, B, 1], FP32, tag="c_e")
                c_p = singles.tile([128, KT, B, 1], FP32, tag="c_p")

                # c_bf doubles as the fp8 scores stationary (ws-scaled)
                c_bf = singles.tile([128, KT, B, 1], GDT, tag="c_bf")
                atth = singles.tile([128, KT, B], GDT, tag="atth")
                inp_bf = singles.tile([128, FT, B], GDT, tag="inp_bf")
                e_bc = singles.tile([128, B * TCAT], BF, tag="e_bc")
                masked32 = singles.tile([B, B * TCAT], BF, tag="masked32")
                scoresbt = singles.tile([B, TCAT], FP32, tag="scoresbt")
                neg_mx = singles.tile([B, 1], FP32, tag="neg_mx")
                e32 = singles.tile([B, TCAT], FP32, tag="e32")
                ssum32 = singles.tile([B, 1], FP32, tag="ssum32")
                rs32 = singles.tile([B, 1], FP32, tag="rs32")
                attw32 = singles.tile([B, TCAT], BF, tag="attw32")
                aw_m = singles.tile([B, B * TCAT], BF, tag="aw_m")
                prod2 = singles.tile([128, KT // 2, B * TCAT], BF,
                                     tag="prod2")
                oT_sb = singles.tile([P, T_OUT * B], FP32, tag="oT_sb")
                # fp8 mirror of cat (ws-scaled): recurrent matmul moving,
                # scores moving, and context source
                cat8 = None
                if gate_fp8:
                    cat8 = singles.tile([128, KT, B, TCAT], F8, tag="cat8")

                XDT = GDT
                xf = singles.tile([128, FT, T_IN * B], XDT, tag="xf")
                zf = singles.tile([128, FT, T_IN * B], XDT, tag="zf")

                wih3_e = wih_e.rearrange("p (kt g) -> p kt g", g=G)
                whh3_e = whh_e.rearrange("p (kt g) -> p kt g", g=G)
                whh3_p = whh_p.rearrange("p (kt g) -> p kt g", g=G)
                lin3 = linT.rearrange("p (kt f) -> p kt f", f=F)

                # ------------- ToFeature --------------------------------
                def to_feature(src, dst):
                    for ft in range(FT):
                        ps = mps_pool.tile([128, T_IN * B], FP32, tag="mps")
                        nc.tensor.matmul(ps, tfT[:, ft * 128:(ft + 1) * 128],
                                         src[:, :], start=True,
                                         stop=not has_btf)
                        if has_btf:
                            nc.tensor.matmul(
                                ps,
                                bias_sb["b_tf"][0:1, ft * 128:(ft + 1) * 128],
                                ones_n[0:1, :], start=False, stop=True)
                        cast_scaled(dst[:, ft, :], ps, ws)

                to_feature(xT, xf)
                to_feature(zT, zf)

                # ------------- encoder gates_x precompute ----------------
                # gx layout: (128, T_IN, MT, B) so per-step slices are
                # contiguous.
                def gates_x(wih3, src, dst, bkey):
                    for mt in range(MT):
                        ps = mps_pool.tile([128, T_IN * B], FP32, tag="mps")
                        k_loop(ps, wih3[:, :, mt * 128:(mt + 1) * 128],
                               src, FT, True, bkey is None, pm=PM)
                        if bkey is not None:
                            nc.tensor.matmul(
                                ps, bias_sb[bkey][0:1, mt * 128:(mt + 1) * 128],
                                ones_n[0:1, :], start=False, stop=True)
                        nc.vector.tensor_copy(
                            out=dst[:, :, mt, :],
                            in_=ps.rearrange("p (t b) -> p t b", b=B))

                gx_e = gx_pool.tile([128, T_IN, MT, B], BF, tag="gx")
                gates_x(wih3_e, xf, gx_e, "b_e" if has_be else None)

                wih3_p = wih_p.rearrange("p (kt g) -> p kt g", g=G)
                gx_p = gx_pool.tile([128, T_IN, MT, B], BF, tag="gx")
                gates_x(wih3_p, zf, gx_p, "b_p" if has_bp else None)

                wih_dd = wih_pool.tile([128, FT * G], GDT, tag="wih")
                for kt in range(FT):
                    nc.sync.dma_start(out=wih_dd[:, kt * G:(kt + 1) * G],
                                      in_=wih_d_d["d"][:, kt * G:(kt + 1) * G])
                wih3_d = wih_dd.rearrange("p (kt g) -> p kt g", g=G)

                def pe_filler(dep_ap):
                    # Tiny matmul dependent on a just-produced DVE/ACT tile.
                    # Keeps the PE HAM activity window busy during long
                    # DVE/ACT chains so the next real matmul block starts at
                    # 2.4 GHz instead of re-warming from 1.2 GHz.
                    if not warm_fillers:
                        return
                    fps = mps_pool.tile([1, 8], FP32, tag="mps")
                    nc.tensor.matmul(fps[:, 0:1], dep_ap, dep_ap,
                                     start=True, stop=True)

                # ------------- LSTM gate elementwise ---------------------
                # gates packed [i, f, o, g]: one Sigmoid over [0:3H], one
                # Tanh over g; ACT never does copies.
                def lstm_tail(gsrc, c_tile, h_out, first_step,
                              emit_cbf=False, h8_out=None):
                    sio = ew_pool.tile([128, 3 * KT, B], FP32, tag="ew")
                    tg = ew_pool.tile([128, KT, B], FP32, tag="ew")
                    nc.scalar.activation(
                        out=sio, in_=gsrc[:, 0:3 * KT, :],
                        func=mybir.ActivationFunctionType.Sigmoid,
                        scale=gsc)
                    nc.scalar.activation(
                        out=tg, in_=gsrc[:, 3 * KT:4 * KT, :],
                        func=mybir.ActivationFunctionType.Tanh,
                        scale=gsc)
                    si = sio[:, 0:KT, :]
                    sf = sio[:, KT:2 * KT, :]
                    so = sio[:, 2 * KT:3 * KT, :]
                    cs = c_tile[:, :, :, 0]
                    pe_filler(sio[:, 0:1, 0:1])
                    # tg <- i*g
                    nc.vector.tensor_mul(tg, si, tg)
                    if first_step:
                        nc.vector.tensor_copy(out=cs, in_=tg)
                    else:
                        nc.vector.tensor_mul(cs, sf, cs)
                        nc.vector.tensor_add(cs, cs, tg)
                    th = ew_pool.tile([128, KT, B], FP32, tag="ew")
                    nc.scalar.activation(
                        out=th, in_=cs,
                        func=mybir.ActivationFunctionType.Tanh)
                    if emit_cbf:
                        # next decoder step's attention stationary, cast
                        # while ACT computes tanh / h
                        cast_scaled(c_bf, c_tile, ws)
                    nc.vector.tensor_mul(h_out, so, th)
                    if h8_out is not None:
                        cast_scaled(h8_out, h_out, ws)

                # ------------- encoder chains (interleaved) --------------
                def enc_step(whh3, gx, c_tile, slot, t, gpool):
                    h_out = cat[:, :, :, slot]
                    h8 = cat8[:, :, :, slot] if gate_fp8 else None
                    if t == 0:
                        lstm_tail(gx[:, 0, :, :], c_tile, h_out, True,
                                  h8_out=h8)
                        return
                    gps = gpool.tile([128, MT, B], FP32, tag="g")
                    # init psum with gx[t] via two wide identity matmuls.
                    # PSUM start_tensor_calc marks a whole 2KB bank pending-
                    # zero, so exactly ONE matmul per bank may carry
                    # start=True; every other matmul accumulates (first
                    # touch of a pending byte auto-replaces).
                    half = MT // 2
                    for ci in range(2):
                        nc.tensor.matmul(
                            gps[:, ci * half:(ci + 1) * half, :], ident,
                            gx[:, t, ci * half:(ci + 1) * half, :],
                            start=True, stop=False)
                    prev = (cat8 if gate_fp8 else cat)[:, :, :, slot - 1]
                    for mt in range(MT):
                        k_loop(gps[:, mt, :],
                               whh3[:, :, mt * 128:(mt + 1) * 128],
                               prev, KT, False,
                               mt in (MT // 2 - 1, MT - 1))
                    lstm_tail(gps, c_tile, h_out, False, h8_out=h8)

                if "enc" not in ablate:
                    for t in range(T_IN):
                        enc_step(whh3_e, gx_e, c_e, t, t, pB_pool)
                        enc_step(whh3_p, gx_p, c_p, T_IN + 1 + t, t, pC_pool)
                else:
                    nc.vector.memset(cat, 0.01)
                    nc.vector.memset(c_e, 0.01)
                    nc.vector.memset(c_p, 0.01)
                    if gate_fp8:
                        nc.vector.memset(cat8, 0.01)

                # ------------- decoder ----------------------------------
                whh_dd = whh_pool.tile([128, KT * G], GDT, tag="whh")
                for kt in range(KT):
                    nc.sync.dma_start(out=whh_dd[:, kt * G:(kt + 1) * G],
                                      in_=whh_d_d["d"][:, kt * G:(kt + 1) * G])
                whh3_d = whh_dd.rearrange("p (kt g) -> p kt g", g=G)

                dec_hs = gx_pool.tile([128, KT, T_OUT, B], BF, tag="gx")

                nc.vector.tensor_copy(out=cat[:, :, :, SLOT_DEC],
                                      in_=cat[:, :, :, T_IN - 1])
                c_d = c_e
                cast_scaled(c_bf, c_d, ws)
                if gate_fp8:
                    nc.vector.tensor_copy(out=cat8[:, :, :, SLOT_DEC],
                                          in_=cat8[:, :, :, T_IN - 1])

                nhalf = B // 2
                h_dec = cat[:, :, :, SLOT_DEC]
                h_mov = (cat8 if gate_fp8 else cat)[:, :, :, SLOT_DEC]

                dec_steps = 0 if "dec" in ablate else T_OUT
                for t in range(dec_steps):
                    # lin: inp = h @ lin_w.T
                    ips = mps_pool.tile([128, FT, B], FP32, tag="mps")
                    for mt in range(FT):
                        k_loop(ips[:, mt, :],
                               lin3[:, :, mt * 128:(mt + 1) * 128],
                               h_mov, KT, True, not has_blin)
                        if has_blin:
                            nc.tensor.matmul(
                                ips[:, mt, :],
                                bias_sb["b_lin"][0:1, mt * 128:(mt + 1) * 128],
                                ones_n[0:1, 0:B], start=False, stop=True)
                    cast_scaled(inp_bf, ips, 1.0 / ws)

                    # gates psum: wih part first (runs under the softmax
                    # chain), whh part accumulates directly afterwards.
                    gps = (pB_pool if t % 2 == 0 else pC_pool).tile(
                        [128, MT, B], FP32, tag="g")

                    def emit_gpw():
                        # one start=True per 2KB psum bank (mt 0 and 16);
                        # all other writes rely on pending-zero auto-replace
                        for mt in range(MT):
                            k_loop(gps[:, mt, :],
                                   wih3_d[:, :, mt * 128:(mt + 1) * 128],
                                   inp_bf, FT, mt in (0, MT // 2), False)
                            if has_bd:
                                nc.tensor.matmul(
                                    gps[:, mt, :],
                                    bias_sb["b_d"][0:1,
                                                   mt * 128:(mt + 1) * 128],
                                    ones_n[0:1, 0:B], start=False, stop=False)

                    if t == 0 and "attn" in ablate:
                        nc.vector.memset(atth, 0.01)
                    if "attn" in ablate:
                        emit_gpw()
                    if "attn" not in ablate:
                        # scores via diagonal matmul: out[b,(b',t)] =
                        # sum_h c[h,b] cat[h,b',t]; diagonal extracted by
                        # mask-multiply + strided reduce. Scores land
                        # batch-on-partitions -> softmax is 4 cheap ops.
                        scd = pA_pool.tile([B, 2, 512], FP32, tag="pA")
                        if PM is not None:
                            for kp in range(KT // 2):
                                sl = slice(2 * kp, 2 * kp + 2)
                                nc.tensor.matmul(
                                    scd[:, 0, 0:nhalf * TCAT],
                                    c_bf[:, sl, :, 0],
                                    cat8[:, sl, 0:nhalf, :],
                                    start=(kp == 0), stop=(kp == KT // 2 - 1),
                                    perf_mode=PM)
                                nc.tensor.matmul(
                                    scd[:, 1, 0:nhalf * TCAT],
                                    c_bf[:, sl, :, 0],
                                    cat8[:, sl, nhalf:B, :],
                                    start=(kp == 0), stop=(kp == KT // 2 - 1),
                                    perf_mode=PM)
                        else:
                            sc_mov = cat8 if gate_fp8 else cat
                            for kt in range(KT):
                                nc.tensor.matmul(
                                    scd[:, 0, 0:nhalf * TCAT],
                                    c_bf[:, kt, :, 0],
                                    sc_mov[:, kt, 0:nhalf, :],
                                    start=(kt == 0), stop=(kt == KT - 1))
                                nc.tensor.matmul(
                                    scd[:, 1, 0:nhalf * TCAT],
                                    c_bf[:, kt, :, 0],
                                    sc_mov[:, kt, nhalf:B, :],
                                    start=(kt == 0), stop=(kt == KT - 1))
                        emit_gpw()
                        nc.vector.tensor_mul(
                            masked32.rearrange("p (c n) -> p c n", c=2),
                            scd[:, :, 0:nhalf * TCAT],
                            dmask.rearrange("p (c n) -> p c n", c=2))
                        nc.vector.tensor_reduce(
                            scoresbt,
                            masked32.rearrange("p (b t) -> p t b", t=TCAT),
                            axis=mybir.AxisListType.X, op=mybir.AluOpType.add)
                        pe_filler(scoresbt[:, 0:1])
                        nc.vector.tensor_reduce(
                            neg_mx, scoresbt, axis=mybir.AxisListType.X,
                            op=mybir.AluOpType.max, negate=True)
                        if gsc != 1.0:
                            # scores psum carries ws^2; exp descales via its
                            # scale arg, the (post-scale) bias needs the same
                            nc.vector.tensor_scalar_mul(neg_mx, neg_mx, gsc)
                        nc.scalar.activation(
                            out=e32, in_=scoresbt,
                            func=mybir.ActivationFunctionType.Exp,
                            bias=neg_mx, scale=gsc, accum_out=ssum32)
                        pe_filler(e32[:, 0:1])
                        nc.vector.reciprocal(rs32, ssum32)
                        nc.vector.tensor_scalar_mul(attw32, e32, rs32)
                        # broadcast attw to all partitions without a DMA:
                        # aw_m[b, (b', t)] = attw[b, t] * dmask, then a K=32
                        # ones matmul sums over b leaving attw[b', t]
                        # replicated on every partition.
                        # broadcast attw to all partitions without a DMA:
                        # aw_m[b, (b', t)] = attw[b, t] * dmask, then a K=32
                        # ones matmul sums over b leaving attw[b', t]
                        # replicated on every partition.
                        nc.vector.tensor_mul(
                            aw_m, dmask,
                            attw32.unsqueeze(1).to_broadcast((B, B, TCAT)))
                        eall = pA_pool.tile([128, 2, 512], FP32, tag="pA")
                        nc.tensor.matmul(eall[:, 0, 0:nhalf * TCAT],
                                         ones32, aw_m[:, 0:nhalf * TCAT],
                                         start=True, stop=True)
                        nc.tensor.matmul(eall[:, 1, 0:nhalf * TCAT],
                                         ones32, aw_m[:, nhalf * TCAT:],
                                         start=True, stop=True)
                        nc.vector.tensor_copy(
                            out=e_bc.rearrange("p (c n) -> p c n", c=2),
                            in_=eall[:, :, 0:nhalf * TCAT])

                        # context: atth[:, kt, :] = sum_t cat[:, kt] * attw
                        # (from cat8 in fp8 mode, so ctx lands ws-scaled and
                        # atth is a plain copy). Two 4-kt chunks: 3 wide DVE
                        # ops each instead of 24 narrow ones, while the whh
                        # k-loop still unblocks after the first chunk.
                        ctx_src = cat8 if gate_fp8 else cat
                        ctx = ew_pool.tile([128, KT, B], FP32, tag="ew")
                        kh = KT // 2
                        e_bc4 = (e_bc.rearrange("p (b t) -> p b t", t=TCAT)
                                 .unsqueeze(1).to_broadcast((128, kh, B, TCAT)))
                        for ci in range(2):
                            ks = slice(ci * kh, (ci + 1) * kh)
                            pr = prod2.rearrange("p k (b t) -> p k b t",
                                                 t=TCAT)
                            nc.vector.tensor_mul(
                                pr, ctx_src[:, ks, :, :], e_bc4)
                            nc.vector.tensor_reduce(
                                ctx[:, ks, :], pr,
                                axis=mybir.AxisListType.X,
                                op=mybir.AluOpType.add)
                            nc.vector.tensor_copy(out=atth[:, ks, :],
                                                  in_=ctx[:, ks, :])
                            pe_filler(ctx[:, ci * kh:ci * kh + 1, 0:1])

                    # whh accumulates into the same psum, k(-pair) outer so
                    # the first pairs start while later context slices are
                    # still reducing on DVE.
                    for kt in range(KT):
                        for mt in range(MT):
                            nc.tensor.matmul(
                                gps[:, mt, :],
                                whh3_d[:, kt, mt * 128:(mt + 1) * 128],
                                atth[:, kt, :],
                                start=False,
                                stop=(kt == KT - 1
                                      and mt in (MT // 2 - 1, MT - 1)))

                    lstm_tail(gps, c_d, h_dec, False,
                              emit_cbf=(t < T_OUT - 1),
                              h8_out=(cat8[:, :, :, SLOT_DEC]
                                      if gate_fp8 else None))
                    nc.vector.tensor_copy(out=dec_hs[:, :, t, :],
                                          in_=h_dec)

                # ------------- ToPose + residual ------------------------
                if "dec" in ablate:
                    return
                ops = pA_pool.tile([P, 2, 512], FP32, tag="pA")
                chunks = [(0, 13), (13, 12)]
                for ci, (t0, tn) in enumerate(chunks):
                    n = tn * B
                    for kt in range(KT):
                        nc.tensor.matmul(
                            ops[:, ci, 0:n],
                            tpT[:, kt * P:(kt + 1) * P],
                            dec_hs[:, kt, t0:t0 + tn, :].rearrange(
                                "p t b -> p (t b)"),
                            start=(kt == 0),
                            stop=(kt == KT - 1 and not has_btp))
                    if has_btp:
                        nc.tensor.matmul(
                            ops[:, ci, 0:n], bias_sb["b_tp"][0:1, :],
                            ones_n[0:1, 0:n], start=False, stop=True)
                    nc.vector.tensor_add(
                        oT_sb[:, t0 * B:t0 * B + n],
                        ops[:, ci, 0:n],
                        residT[:, t0 * B:t0 * B + n])
                nc.sync.dma_start(out=out_d[:, :], in_=oT_sb)

            if loop_iters > 1:
                with tc.For_i(0, loop_iters, 1, name="rep"):
                    body()
            else:
                body()

    return nc


# ------------------------------------------------------------- entry point

_model_cache = {}


def _get_model(key):
    if key not in _model_cache:
        bias_flags, gate_fp8 = key
        _model_cache[key] = build_model(bias_flags, gate_fp8=gate_fp8)
    return _model_cache[key]


def make_in_maps(inputs, gate_fp8=GATE_FP8):
    """Host-side packing: returns per-core input maps."""
    w = _prep_weights(inputs, gate_fp8)
    flags = _bias_flags(w)
    x = np.asarray(inputs["x"], dtype=np.float32)
    z = np.asarray(inputs["z"], dtype=np.float32)
    fr = np.asarray(inputs["for_resid"], dtype=np.float32)

    dmask = np.zeros((B, B, TCAT), dtype=np.float32)
    for b in range(B):
        dmask[b, b, :] = 1.0
    shared = {
        "tfT": w["tfT"], "linT": w["linT"], "tpT": w["tpT"],
        "dmask": np.ascontiguousarray(
            dmask.reshape(B, B * TCAT)).astype(BF16),
    }
    for nm in ("e", "p", "d"):
        shared[f"wih_{nm}"] = w[f"wih_{nm}"]
        shared[f"whh_{nm}"] = w[f"whh_{nm}"]
    names = ("b_tf", "b_e", "b_p", "b_d", "b_lin", "b_tp")
    for f, name in zip(flags, names):
        if f:
            shared[name] = np.ascontiguousarray(
                w[name][None, :]).astype(BF16)

    in_maps = []
    for c in range(N_CORES):
        sl = slice(c * B, (c + 1) * B)
        m = dict(shared)
        m["xT"] = np.ascontiguousarray(
            x[sl].transpose(2, 1, 0).reshape(P, T_IN * B)).astype(BF16)
        m["zT"] = np.ascontiguousarray(
            z[sl].transpose(2, 1, 0).reshape(P, T_IN * B)).astype(BF16)
        m["residT"] = np.ascontiguousarray(
            fr[sl].transpose(2, 1, 0).reshape(P, T_OUT * B))
        in_maps.append(m)
    return in_maps, flags


def unshard_output(results):
    outs = []
    for c in range(N_CORES):
        oT = np.asarray(results[c]["oT"])  # (66, 800)
        outs.append(oT.reshape(P, T_OUT, B).transpose(2, 1, 0))
    return np.ascontiguousarray(np.concatenate(outs, axis=0),
                                dtype=np.float32)


def kernel(**inputs) -> np.ndarray:
    in_maps, flags = make_in_maps(inputs)
    nc = _get_model((flags, GATE_FP8))
    res = run_bass_kernel_spmd(nc, in_maps, core_ids=list(range(N_CORES)))
    return unshard_output(res.results)


# revision 51
# speedup vs baseline: 1.0063x; 1.0063x over previous
"""Trainium2 Bass kernel for nn_AttentionModel (seq2seq LSTM with attention).

Sharding: pure data parallelism over batch (256 -> 8 cores x 32), all
weights replicated. Per-core layout keeps the hidden/gate dimension on
SBUF partitions and (time, batch) on the free axis so the recurrent
matmuls, elementwise gate math, and attention all use one consistent
layout with no on-device transposes.

v2 performance structure (the kernel is LDWEIGHTS-bound: per 128x128
weight tile the PE pays ~128 rows of load for only B=32 moving columns):
- all gate weights (whh_e/p/d, wih_e/p/d, linT) stored fp8 e4m3 and
  consumed with DoubleRow matmuls: each instruction contracts a 256-row
  k-pair, halving both weight-load time and instruction count
- decoder wih+whh gates accumulate into ONE psum tile (no identity-
  matmul merge passes, no psum->sbuf->psum round trips)
- encoder per-step gx add is 2 wide identity matmuls (psum init), not
  32 narrow ones
- the two encoder chains are interleaved so one chain's gate/cell
  elementwise tail (ACT/DVE) hides under the other chain's recurrent
  matmuls (PE)
- gates packed [i, f, o, g] so one Sigmoid covers i,f,o contiguously
- attention scores via a diagonal matmul (c stationary) so the softmax
  lands batch-on-partitions and runs as per-partition-scalar ops

Self-contained: includes the TileContext wait-split workaround and all
host-side packing. The graded entry point is kernel(**inputs).
"""

import numpy as np
import ml_dtypes

import concourse.bass as bass
import concourse.mybir as mybir
import concourse.tile as tile
from concourse.bass_isa import ReduceOp
from concourse.bass_utils import run_bass_kernel_spmd

BF16 = ml_dtypes.bfloat16
FP8 = ml_dtypes.float8_e4m3
FP32 = mybir.dt.float32
BF = mybir.dt.bfloat16
F8 = mybir.dt.float8e4

GATE_FP8 = True     # fp8 gate weights (2x faster LDWEIGHTS + DoubleRow)
DOUBLE_ROW = True   # contract k-tile pairs per matmul instruction

N_CORES = 8
B = 32            # batch per core
T_IN = 10
T_OUT = 25
H = 1024
F = 512
P = 66
G = 4 * H         # 4096 gates
KT = H // 128     # 8  k-tiles over hidden
FT = F // 128     # 4  k-tiles over feature
MT = G // 128     # 32 m-tiles over gates
TCAT = 2 * T_IN + 1   # 21 attention slots
SLOT_DEC = T_IN       # decoder h lives at slot 10

_MAX_WAITS = 1


def _apply_tile_wait_patches():
    """The walrus CoreV3 codegen in this container rejects instructions
    carrying more than one sync-wait command ("Too many sync wait
    commands"). Keep every instruction at <=1 wait by moving excess waits
    onto same-engine nops emitted immediately before the instruction."""
    import bass_rust
    from concourse.vector_clock import ScopedClock

    SyncInfo = bass_rust.SyncInfo

    def _split_waits(nc, inst):
        si = getattr(inst, "sync_info", None)
        if si is None or not si.on_wait or len(si.on_wait) <= _MAX_WAITS:
            return
        if inst.engine == mybir.EngineType.Unassigned:
            return
        waits = list(si.on_wait)
        si.on_wait = waits[:_MAX_WAITS]
        rest = waits[_MAX_WAITS:]
        eng = nc.engines[inst.engine]
        for i in range(0, len(rest), _MAX_WAITS):
            nop = eng.nop(nofuse=True, hint="wait_split")
            nop.ins.sync_info = SyncInfo(
                on_wait=rest[i:i + _MAX_WAITS], on_update=[]
            )

    orig_commit = tile.TileContext._commit_instruction

    def _commit_split(self, inst, lazy_reg_writes=True):
        si = getattr(inst, "sync_info", None)
        if (si is not None and si.on_wait is not None
                and len(si.on_wait) > _MAX_WAITS
                and inst.engine != mybir.EngineType.Unassigned):
            _split_waits(self.nc, inst)
        return orig_commit(self, inst, lazy_reg_writes)

    tile.TileContext._commit_instruction = _commit_split

    def _drain_and_barrier_split(self, tick_clock, wait_clock):
        drain_inst = self.nc.sync.drain()
        wait_clock.add_sem_waits(
            drain_inst.ins, ScopedClock({None: tick_clock.global_clock})
        )
        sync_info = drain_inst.ins.sync_info
        if sync_info is not None and sync_info.on_wait is not None:
            waits = list(sync_info.on_wait)
            if len(waits) > _MAX_WAITS:
                sync_info.on_wait = waits[:_MAX_WAITS]
                rest = waits[_MAX_WAITS:]
                for i in range(0, len(rest), _MAX_WAITS):
                    nop = self.nc.sync.nop(nofuse=True, hint="drain_wait_split")
                    nop.ins.sync_info = SyncInfo(
                        on_wait=rest[i:i + _MAX_WAITS], on_update=[]
                    )
        self.nc.all_engine_barrier()
        assert self.sems is not None
        popped = self.nc._tile_sem_poison_stack.pop()
        assert popped is self._sem_poison
        self.nc.clear_and_free_semaphores(list(self.sems.allocated().values()))
        self.nc.all_engine_barrier()

    tile.TileContext._drain_and_barrier = _drain_and_barrier_split


_apply_tile_wait_patches()

# Opt-in: flip walrus --enable-ldw-opt (hardcoded false in
# bass_utils.bir_verify_and_optimise) by rewriting the command line.
LDW_OPT = False


def _apply_ldw_opt_patch():
    from concourse import bass_utils as _bu
    if getattr(_bu, "_ldw_opt_patched", False):
        return
    _bu._ldw_opt_patched = True
    _orig_run_command = _bu.run_command

    def _run_command_ldw(argv, **kwargs):
        if LDW_OPT:
            argv = ["--enable-ldw-opt=true" if a == "--enable-ldw-opt=false"
                    else a for a in argv]
        return _orig_run_command(argv, **kwargs)

    _bu.run_command = _run_command_ldw


_apply_ldw_opt_patch()


# ------------------------------------------------------------- host packing

# gate reorder: reference packs gates [i, f, g, o]; we use [i, f, o, g]
# so the three sigmoids (i, f, o) are contiguous.
_GPERM = np.concatenate([
    np.arange(0, H),              # i
    np.arange(H, 2 * H),          # f
    np.arange(3 * H, 4 * H),      # o
    np.arange(2 * H, 3 * H),      # g
])


def _pack_T(w, ktiles, mcols, dt=BF16):
    """(mcols, ktiles*128) weight -> transposed tiled layout
    (128, ktiles*mcols) with [p, kt*mcols + m] = w[m, kt*128 + p]."""
    wT = np.ascontiguousarray(np.asarray(w, np.float32).T).astype(dt)
    return np.ascontiguousarray(
        wT.reshape(ktiles, 128, mcols).transpose(1, 0, 2)
        .reshape(128, ktiles * mcols))


WS = 16.0   # fp8 pre-quantization scale: lifts glorot weights and unit
            # activations out of e4m3's subnormal range (the PE flushes
            # subnormal fp8 operands); gates come out x256, undone by the
            # activation's scale argument.


def _prep_weights(inputs, gate_fp8=GATE_FP8):
    gdt = FP8 if gate_fp8 else BF16
    ws = WS if gate_fp8 else 1.0
    d = {}
    d["tfT"] = np.ascontiguousarray(
        np.asarray(inputs["tf_w"], np.float32).T).astype(BF16)  # (66, 512)
    for nm, wih, whh in (("e", "enc_wih", "enc_whh"),
                         ("p", "encp_wih", "encp_whh"),
                         ("d", "dec_wih", "dec_whh")):
        d[f"wih_{nm}"] = _pack_T(
            np.asarray(inputs[wih], np.float32)[_GPERM] * ws, FT, G, gdt)
        d[f"whh_{nm}"] = _pack_T(
            np.asarray(inputs[whh], np.float32)[_GPERM] * ws, KT, G, gdt)
    d["linT"] = _pack_T(
        np.asarray(inputs["lin_w"], np.float32) * ws, KT, F, gdt)
    d["tpT"] = _pack_T(inputs["tp_w"], KT, P)            # (128, 8*66)
    d["b_tf"] = np.asarray(inputs["tf_b"], np.float32)
    for nm, bi, bh in (("e", "enc_bih", "enc_bhh"),
                       ("p", "encp_bih", "encp_bhh"),
                       ("d", "dec_bih", "dec_bhh")):
        d[f"b_{nm}"] = (np.asarray(inputs[bi], np.float32)
                        + np.asarray(inputs[bh], np.float32))[_GPERM] * ws * ws
    d["b_lin"] = np.asarray(inputs["lin_b"], np.float32) * ws * ws
    d["b_tp"] = np.asarray(inputs["tp_b"], np.float32)
    return d


def _bias_flags(w):
    return tuple(bool(np.any(w[k])) for k in
                 ("b_tf", "b_e", "b_p", "b_d", "b_lin", "b_tp"))


# ------------------------------------------------------------ device build

def build_model(bias_flags=(False,) * 6, loop_iters=1, gate_fp8=GATE_FP8,
                double_row=DOUBLE_ROW, ablate=(), warm_fillers=True):
    has_btf, has_be, has_bp, has_bd, has_blin, has_btp = bias_flags
    any_bias = any(bias_flags)
    GDT = F8 if gate_fp8 else BF
    PM = (mybir.MatmulPerfMode.DoubleRow
          if (gate_fp8 and double_row) else None)

    ws = WS if gate_fp8 else 1.0
    gsc = 1.0 / (ws * ws)   # gates psum carries ws^2; undone inside ACT

    nc = bass.Bass()

    xT_d = nc.dram_tensor("xT", [P, T_IN * B], BF, kind="ExternalInput")
    zT_d = nc.dram_tensor("zT", [P, T_IN * B], BF, kind="ExternalInput")
    residT_d = nc.dram_tensor("residT", [P, T_OUT * B], FP32,
                              kind="ExternalInput")
    tfT_d = nc.dram_tensor("tfT", [P, F], BF, kind="ExternalInput")
    wih_d_d = {}
    whh_d_d = {}
    for nm in ("e", "p", "d"):
        wih_d_d[nm] = nc.dram_tensor(f"wih_{nm}", [128, FT * G], GDT,
                                     kind="ExternalInput")
        whh_d_d[nm] = nc.dram_tensor(f"whh_{nm}", [128, KT * G], GDT,
                                     kind="ExternalInput")
    linT_d = nc.dram_tensor("linT", [128, KT * F], GDT, kind="ExternalInput")
    dmask_d = nc.dram_tensor("dmask", [B, B * TCAT], BF, kind="ExternalInput")
    tpT_d = nc.dram_tensor("tpT", [128, KT * P], BF, kind="ExternalInput")
    bias_d = {}
    if has_btf:
        bias_d["b_tf"] = nc.dram_tensor("b_tf", [1, F], BF,
                                        kind="ExternalInput")
    if has_be:
        bias_d["b_e"] = nc.dram_tensor("b_e", [1, G], BF,
                                       kind="ExternalInput")
    if has_bp:
        bias_d["b_p"] = nc.dram_tensor("b_p", [1, G], BF,
                                       kind="ExternalInput")
    if has_bd:
        bias_d["b_d"] = nc.dram_tensor("b_d", [1, G], BF,
                                       kind="ExternalInput")
    if has_blin:
        bias_d["b_lin"] = nc.dram_tensor("b_lin", [1, F], BF,
                                         kind="ExternalInput")
    if has_btp:
        bias_d["b_tp"] = nc.dram_tensor("b_tp", [1, P], BF,
                                        kind="ExternalInput")
    out_d = nc.dram_tensor("oT", [P, T_OUT * B], FP32, kind="ExternalOutput")

    with tile.TileContext(nc) as tc:
        with (
            tc.tile_pool(name="singles", bufs=1) as singles,
            tc.tile_pool(name="wih_pool", bufs=2) as wih_pool,
            tc.tile_pool(name="whh_pool", bufs=2) as whh_pool,
            tc.tile_pool(name="gx_pool", bufs=2) as gx_pool,
            tc.tile_pool(name="ew_pool", bufs=4) as ew_pool,
            tc.tile_pool(name="pA_pool", bufs=1, space="PSUM") as pA_pool,
            tc.tile_pool(name="pB_pool", bufs=1, space="PSUM") as pB_pool,
            tc.tile_pool(name="pC_pool", bufs=1, space="PSUM") as pC_pool,
            tc.tile_pool(name="mps_pool", bufs=2, space="PSUM") as mps_pool,
        ):
            def body(_it=None):
                def cast_scaled(out, in_, s):
                    # fp8 cast with pre-scale (lifts values out of the
                    # subnormal-flush range of the PE's fp8 datapath)
                    if s == 1.0:
                        nc.vector.tensor_copy(out=out, in_=in_)
                    else:
                        nc.vector.tensor_scalar_mul(out, in_, float(s))

                # k_loop: contract kt-tiles of w3 [128, ktn, M] against
                # mov3 [128, ktn, N] into psum `out`; DoubleRow pairs
                # k-tiles when enabled.
                def k_loop(out, w3, mov3, ktn, first, last, pm=None):
                    # pm=DoubleRow only pays off for wide moving operands:
                    # a DR stationary loads at 1 row/cycle unoverlapped
                    # (256 cyc), while consecutive non-DR matmuls hide the
                    # weight load under the previous matmul's moving phase
                    # (~64-74 cyc for K=128). Narrow (B=32) k-loops must
                    # stay non-DR.
                    if pm is not None:
                        for kp in range(ktn // 2):
                            nc.tensor.matmul(
                                out, w3[:, 2 * kp:2 * kp + 2, :],
                                mov3[:, 2 * kp:2 * kp + 2, :],
                                start=(first and kp == 0),
                                stop=(last and kp == ktn // 2 - 1),
                                perf_mode=pm)
                    else:
                        for kt in range(ktn):
                            nc.tensor.matmul(
                                out, w3[:, kt, :], mov3[:, kt, :],
                                start=(first and kt == 0),
                                stop=(last and kt == ktn - 1))

                # ------------- constant/static loads --------------------
                tfT = singles.tile([P, F], BF, tag="tfT")
                nc.sync.dma_start(out=tfT, in_=tfT_d[:, :])
                xT = singles.tile([P, T_IN * B], BF, tag="xT")
                nc.sync.dma_start(out=xT, in_=xT_d[:, :])
                zT = singles.tile([P, T_IN * B], BF, tag="zT")
                nc.sync.dma_start(out=zT, in_=zT_d[:, :])

                wih_e = wih_pool.tile([128, FT * G], GDT, tag="wih")
                for kt in range(FT):
                    nc.sync.dma_start(out=wih_e[:, kt * G:(kt + 1) * G],
                                      in_=wih_d_d["e"][:, kt * G:(kt + 1) * G])
                wih_p = wih_pool.tile([128, FT * G], GDT, tag="wih")
                for kt in range(FT):
                    nc.sync.dma_start(out=wih_p[:, kt * G:(kt + 1) * G],
                                      in_=wih_d_d["p"][:, kt * G:(kt + 1) * G])
                whh_e = whh_pool.tile([128, KT * G], GDT, tag="whh")
                for kt in range(KT):
                    nc.sync.dma_start(out=whh_e[:, kt * G:(kt + 1) * G],
                                      in_=whh_d_d["e"][:, kt * G:(kt + 1) * G])
                whh_p = whh_pool.tile([128, KT * G], GDT, tag="whh")
                for kt in range(KT):
                    nc.sync.dma_start(out=whh_p[:, kt * G:(kt + 1) * G],
                                      in_=whh_d_d["p"][:, kt * G:(kt + 1) * G])

                residT = singles.tile([P, T_OUT * B], FP32, tag="residT")
                nc.sync.dma_start(out=residT, in_=residT_d[:, :])
                linT = singles.tile([128, KT * F], GDT, tag="linT")
                for kt in range(KT):
                    nc.sync.dma_start(out=linT[:, kt * F:(kt + 1) * F],
                                      in_=linT_d[:, kt * F:(kt + 1) * F])
                tpT = singles.tile([128, KT * P], BF, tag="tpT")
                nc.sync.dma_start(out=tpT, in_=tpT_d[:, :])
                dmask = singles.tile([B, B * TCAT], BF, tag="dmask")
                nc.sync.dma_start(out=dmask, in_=dmask_d[:, :])

                bias_sb = {}
                for key, dram in bias_d.items():
                    t = singles.tile(list(dram.shape), BF, tag=key)
                    nc.sync.dma_start(out=t, in_=dram[:, :])
                    bias_sb[key] = t

                ones32 = singles.tile([B, 128], BF, tag="ones32")
                nc.vector.memset(ones32, 1.0)
                ident = singles.tile([128, 128], BF, tag="ident")
                from concourse.masks import make_identity
                make_identity(nc, ident)
                if any_bias:
                    ones_n = singles.tile([1, T_IN * B], BF, tag="ones_n")
                    nc.vector.memset(ones_n, 1.0)

                cat = singles.tile([128, KT, B, TCAT], BF, tag="cat")
                c_e = singles.tile([128, KT, B, 1], FP32, tag="c_e")
                c_p = singles.tile([128, KT, B, 1], FP32, tag="c_p")

                # c_bf doubles as the fp8 scores stationary (ws-scaled)
                c_bf = singles.tile([128, KT, B, 1], GDT, tag="c_bf")
                atth = singles.tile([128, KT, B], GDT, tag="atth")
                inp_bf = singles.tile([128, FT, B], GDT, tag="inp_bf")
                e_bc = singles.tile([128, B * TCAT], BF, tag="e_bc")
                masked32 = singles.tile([B, B * TCAT], FP32, tag="masked32")
                scoresbt = singles.tile([B, TCAT], FP32, tag="scoresbt")
                neg_mx = singles.tile([B, 1], FP32, tag="neg_mx")
                e32 = singles.tile([B, TCAT], FP32, tag="e32")
                ssum32 = singles.tile([B, 1], FP32, tag="ssum32")
                rs32 = singles.tile([B, 1], FP32, tag="rs32")
                attw32 = singles.tile([B, TCAT], BF, tag="attw32")
                aw_m = singles.tile([B, B * TCAT], BF, tag="aw_m")
                prod2 = singles.tile([128, B * TCAT], BF, tag="prod2")
                oT_sb = singles.tile([P, T_OUT * B], FP32, tag="oT_sb")
                # fp8 mirror of cat (ws-scaled): recurrent matmul moving,
                # scores moving, and context source
                cat8 = None
                if gate_fp8:
                    cat8 = singles.tile([128, KT, B, TCAT], F8, tag="cat8")

                XDT = GDT
                xf = singles.tile([128, FT, T_IN * B], XDT, tag="xf")
                zf = singles.tile([128, FT, T_IN * B], XDT, tag="zf")

                wih3_e = wih_e.rearrange("p (kt g) -> p kt g", g=G)
                whh3_e = whh_e.rearrange("p (kt g) -> p kt g", g=G)
                whh3_p = whh_p.rearrange("p (kt g) -> p kt g", g=G)
                lin3 = linT.rearrange("p (kt f) -> p kt f", f=F)

                # ------------- ToFeature --------------------------------
                def to_feature(src, dst):
                    for ft in range(FT):
                        ps = mps_pool.tile([128, T_IN * B], FP32, tag="mps")
                        nc.tensor.matmul(ps, tfT[:, ft * 128:(ft + 1) * 128],
                                         src[:, :], start=True,
                                         stop=not has_btf)
                        if has_btf:
                            nc.tensor.matmul(
                                ps,
                                bias_sb["b_tf"][0:1, ft * 128:(ft + 1) * 128],
                                ones_n[0:1, :], start=False, stop=True)
                        cast_scaled(dst[:, ft, :], ps, ws)

                to_feature(xT, xf)
                to_feature(zT, zf)

                # ------------- encoder gates_x precompute ----------------
                # gx layout: (128, T_IN, MT, B) so per-step slices are
                # contiguous.
                def gates_x(wih3, src, dst, bkey):
                    for mt in range(MT):
                        ps = mps_pool.tile([128, T_IN * B], FP32, tag="mps")
                        k_loop(ps, wih3[:, :, mt * 128:(mt + 1) * 128],
                               src, FT, True, bkey is None, pm=PM)
                        if bkey is not None:
                            nc.tensor.matmul(
                                ps, bias_sb[bkey][0:1, mt * 128:(mt + 1) * 128],
                                ones_n[0:1, :], start=False, stop=True)
                        nc.vector.tensor_copy(
                            out=dst[:, :, mt, :],
                            in_=ps.rearrange("p (t b) -> p t b", b=B))

                gx_e = gx_pool.tile([128, T_IN, MT, B], BF, tag="gx")
                gates_x(wih3_e, xf, gx_e, "b_e" if has_be else None)

                wih3_p = wih_p.rearrange("p (kt g) -> p kt g", g=G)
                gx_p = gx_pool.tile([128, T_IN, MT, B], BF, tag="gx")
                gates_x(wih3_p, zf, gx_p, "b_p" if has_bp else None)

                wih_dd = wih_pool.tile([128, FT * G], GDT, tag="wih")
                for kt in range(FT):
                    nc.sync.dma_start(out=wih_dd[:, kt * G:(kt + 1) * G],
                                      in_=wih_d_d["d"][:, kt * G:(kt + 1) * G])
                wih3_d = wih_dd.rearrange("p (kt g) -> p kt g", g=G)

                def pe_filler(dep_ap):
                    # Tiny matmul dependent on a just-produced DVE/ACT tile.
                    # Keeps the PE HAM activity window busy during long
                    # DVE/ACT chains so the next real matmul block starts at
                    # 2.4 GHz instead of re-warming from 1.2 GHz.
                    if not warm_fillers:
                        return
                    fps = mps_pool.tile([1, 8], FP32, tag="mps")
                    nc.tensor.matmul(fps[:, 0:1], dep_ap, dep_ap,
                                     start=True, stop=True)

                # ------------- LSTM gate elementwise ---------------------
                # gates packed [i, f, o, g]: one Sigmoid over [0:3H], one
                # Tanh over g; ACT never does copies.
                def lstm_tail(gsrc, c_tile, h_out, first_step,
                              emit_cbf=False, h8_out=None):
                    sio = ew_pool.tile([128, 3 * KT, B], FP32, tag="ew")
                    tg = ew_pool.tile([128, KT, B], FP32, tag="ew")
                    nc.scalar.activation(
                        out=sio, in_=gsrc[:, 0:3 * KT, :],
                        func=mybir.ActivationFunctionType.Sigmoid,
                        scale=gsc)
                    nc.scalar.activation(
                        out=tg, in_=gsrc[:, 3 * KT:4 * KT, :],
                        func=mybir.ActivationFunctionType.Tanh,
                        scale=gsc)
                    si = sio[:, 0:KT, :]
                    sf = sio[:, KT:2 * KT, :]
                    so = sio[:, 2 * KT:3 * KT, :]
                    cs = c_tile[:, :, :, 0]
                    pe_filler(sio[:, 0:1, 0:1])
                    # tg <- i*g
                    nc.vector.tensor_mul(tg, si, tg)
                    if first_step:
                        nc.vector.tensor_copy(out=cs, in_=tg)
                    else:
                        nc.vector.tensor_mul(cs, sf, cs)
                        nc.vector.tensor_add(cs, cs, tg)
                    th = ew_pool.tile([128, KT, B], FP32, tag="ew")
                    nc.scalar.activation(
                        out=th, in_=cs,
                        func=mybir.ActivationFunctionType.Tanh)
                    if emit_cbf:
                        # next decoder step's attention stationary, cast
                        # while ACT computes tanh / h
                        cast_scaled(c_bf, c_tile, ws)
                    nc.vector.tensor_mul(h_out, so, th)
                    if h8_out is not None:
                        cast_scaled(h8_out, h_out, ws)

                # ------------- encoder chains (interleaved) --------------
                def enc_step(whh3, gx, c_tile, slot, t, gpool):
                    h_out = cat[:, :, :, slot]
                    h8 = cat8[:, :, :, slot] if gate_fp8 else None
                    if t == 0:
                        lstm_tail(gx[:, 0, :, :], c_tile, h_out, True,
                                  h8_out=h8)
                        return
                    gps = gpool.tile([128, MT, B], FP32, tag="g")
                    # init psum with gx[t] via two wide identity matmuls.
                    # PSUM start_tensor_calc marks a whole 2KB bank pending-
                    # zero, so exactly ONE matmul per bank may carry
                    # start=True; every other matmul accumulates (first
                    # touch of a pending byte auto-replaces).
                    half = MT // 2
                    for ci in range(2):
                        nc.tensor.matmul(
                            gps[:, ci * half:(ci + 1) * half, :], ident,
                            gx[:, t, ci * half:(ci + 1) * half, :],
                            start=True, stop=False)
                    prev = (cat8 if gate_fp8 else cat)[:, :, :, slot - 1]
                    for mt in range(MT):
                        k_loop(gps[:, mt, :],
                               whh3[:, :, mt * 128:(mt + 1) * 128],
                               prev, KT, False,
                               mt in (MT // 2 - 1, MT - 1))
                    lstm_tail(gps, c_tile, h_out, False, h8_out=h8)

                if "enc" not in ablate:
                    for t in range(T_IN):
                        enc_step(whh3_e, gx_e, c_e, t, t, pB_pool)
                        enc_step(whh3_p, gx_p, c_p, T_IN + 1 + t, t, pC_pool)
                else:
                    nc.vector.memset(cat, 0.01)
                    nc.vector.memset(c_e, 0.01)
                    nc.vector.memset(c_p, 0.01)
                    if gate_fp8:
                        nc.vector.memset(cat8, 0.01)

                # ------------- decoder ----------------------------------
                whh_dd = whh_pool.tile([128, KT * G], GDT, tag="whh")
                for kt in range(KT):
                    nc.sync.dma_start(out=whh_dd[:, kt * G:(kt + 1) * G],
                                      in_=whh_d_d["d"][:, kt * G:(kt + 1) * G])
                whh3_d = whh_dd.rearrange("p (kt g) -> p kt g", g=G)

                dec_hs = gx_pool.tile([128, KT, T_OUT, B], BF, tag="gx")

                nc.vector.tensor_copy(out=cat[:, :, :, SLOT_DEC],
                                      in_=cat[:, :, :, T_IN - 1])
                c_d = c_e
                cast_scaled(c_bf, c_d, ws)
                if gate_fp8:
                    nc.vector.tensor_copy(out=cat8[:, :, :, SLOT_DEC],
                                          in_=cat8[:, :, :, T_IN - 1])

                nhalf = B // 2
                h_dec = cat[:, :, :, SLOT_DEC]
                h_mov = (cat8 if gate_fp8 else cat)[:, :, :, SLOT_DEC]

                dec_steps = 0 if "dec" in ablate else T_OUT
                for t in range(dec_steps):
                    # lin: inp = h @ lin_w.T
                    ips = mps_pool.tile([128, FT, B], FP32, tag="mps")
                    for mt in range(FT):
                        k_loop(ips[:, mt, :],
                               lin3[:, :, mt * 128:(mt + 1) * 128],
                               h_mov, KT, True, not has_blin)
                        if has_blin:
                            nc.tensor.matmul(
                                ips[:, mt, :],
                                bias_sb["b_lin"][0:1, mt * 128:(mt + 1) * 128],
                                ones_n[0:1, 0:B], start=False, stop=True)
                    cast_scaled(inp_bf, ips, 1.0 / ws)

                    # gates psum: wih part first (runs under the softmax
                    # chain), whh part accumulates directly afterwards.
                    gps = (pB_pool if t % 2 == 0 else pC_pool).tile(
                        [128, MT, B], FP32, tag="g")

                    def emit_gpw():
                        # one start=True per 2KB psum bank (mt 0 and 16);
                        # all other writes rely on pending-zero auto-replace
                        for mt in range(MT):
                            k_loop(gps[:, mt, :],
                                   wih3_d[:, :, mt * 128:(mt + 1) * 128],
                                   inp_bf, FT, mt in (0, MT // 2), False)
                            if has_bd:
                                nc.tensor.matmul(
                                    gps[:, mt, :],
                                    bias_sb["b_d"][0:1,
                                                   mt * 128:(mt + 1) * 128],
                                    ones_n[0:1, 0:B], start=False, stop=False)

                    if t == 0 and "attn" in ablate:
                        nc.vector.memset(atth, 0.01)
                    if "attn" in ablate:
                        emit_gpw()
                    if "attn" not in ablate:
                        # scores via diagonal matmul: out[b,(b',t)] =
                        # sum_h c[h,b] cat[h,b',t]; diagonal extracted by
                        # mask-multiply + strided reduce. Scores land
                        # batch-on-partitions -> softmax is 4 cheap ops.
                        scd = pA_pool.tile([B, 2, 512], FP32, tag="pA")
                        if PM is not None:
                            for kp in range(KT // 2):
                                sl = slice(2 * kp, 2 * kp + 2)
                                nc.tensor.matmul(
                                    scd[:, 0, 0:nhalf * TCAT],
                                    c_bf[:, sl, :, 0],
                                    cat8[:, sl, 0:nhalf, :],
                                    start=(kp == 0), stop=(kp == KT // 2 - 1),
                                    perf_mode=PM)
                                nc.tensor.matmul(
                                    scd[:, 1, 0:nhalf * TCAT],
                                    c_bf[:, sl, :, 0],
                                    cat8[:, sl, nhalf:B, :],
                                    start=(kp == 0), stop=(kp == KT // 2 - 1),
                                    perf_mode=PM)
                        else:
                            sc_mov = cat8 if gate_fp8 else cat
                            for kt in range(KT):
                                nc.tensor.matmul(
                                    scd[:, 0, 0:nhalf * TCAT],
                                    c_bf[:, kt, :, 0],
                                    sc_mov[:, kt, 0:nhalf, :],
                                    start=(kt == 0), stop=(kt == KT - 1))
                                nc.tensor.matmul(
                                    scd[:, 1, 0:nhalf * TCAT],
                                    c_bf[:, kt, :, 0],
                                    sc_mov[:, kt, nhalf:B, :],
                                    start=(kt == 0), stop=(kt == KT - 1))
                        emit_gpw()
                        nc.vector.tensor_mul(
                            masked32.rearrange("p (c n) -> p c n", c=2),
                            scd[:, :, 0:nhalf * TCAT],
                            dmask.rearrange("p (c n) -> p c n", c=2))
                        nc.vector.tensor_reduce(
                            scoresbt,
                            masked32.rearrange("p (b t) -> p t b", t=TCAT),
                            axis=mybir.AxisListType.X, op=mybir.AluOpType.add)
                        pe_filler(scoresbt[:, 0:1])
                        nc.vector.tensor_reduce(
                            neg_mx, scoresbt, axis=mybir.AxisListType.X,
                            op=mybir.AluOpType.max, negate=True)
                        if gsc != 1.0:
                            # scores psum carries ws^2; exp descales via its
                            # scale arg, the (post-scale) bias needs the same
                            nc.vector.tensor_scalar_mul(neg_mx, neg_mx, gsc)
                        nc.scalar.activation(
                            out=e32, in_=scoresbt,
                            func=mybir.ActivationFunctionType.Exp,
                            bias=neg_mx, scale=gsc, accum_out=ssum32)
                        pe_filler(e32[:, 0:1])
                        nc.vector.reciprocal(rs32, ssum32)
                        nc.vector.tensor_scalar_mul(attw32, e32, rs32)
                        # broadcast attw to all partitions without a DMA:
                        # aw_m[b, (b', t)] = attw[b, t] * dmask, then a K=32
                        # ones matmul sums over b leaving attw[b', t]
                        # replicated on every partition.
                        # broadcast attw to all partitions without a DMA:
                        # aw_m[b, (b', t)] = attw[b, t] * dmask, then a K=32
                        # ones matmul sums over b leaving attw[b', t]
                        # replicated on every partition.
                        nc.vector.tensor_mul(
                            aw_m, dmask,
                            attw32.unsqueeze(1).to_broadcast((B, B, TCAT)))
                        eall = pA_pool.tile([128, 2, 512], FP32, tag="pA")
                        nc.tensor.matmul(eall[:, 0, 0:nhalf * TCAT],
                                         ones32, aw_m[:, 0:nhalf * TCAT],
                                         start=True, stop=True)
                        nc.tensor.matmul(eall[:, 1, 0:nhalf * TCAT],
                                         ones32, aw_m[:, nhalf * TCAT:],
                                         start=True, stop=True)
                        nc.vector.tensor_copy(out=e_bc[:, 0:nhalf * TCAT],
                                              in_=eall[:, 0, 0:nhalf * TCAT])
                        nc.vector.tensor_copy(out=e_bc[:, nhalf * TCAT:],
                                              in_=eall[:, 1, 0:nhalf * TCAT])

                        # context: atth[:, kt, :] = sum_t cat[:, kt] * attw
                        # (from cat8 in fp8 mode, so ctx lands ws-scaled and
                        # atth is a plain copy)
                        ctx_src = cat8 if gate_fp8 else cat
                        ctx = ew_pool.tile([128, KT, B], FP32, tag="ew")
                        for kt in range(KT):
                            nc.vector.tensor_mul(
                                prod2, ctx_src[:, kt, :, :],
                                e_bc.rearrange("p (b t) -> p b t", t=TCAT))
                            nc.vector.tensor_reduce(
                                ctx[:, kt, :],
                                prod2.rearrange("p (b t) -> p b t", t=TCAT),
                                axis=mybir.AxisListType.X,
                                op=mybir.AluOpType.add)
                            nc.vector.tensor_copy(out=atth[:, kt, :],
                                                  in_=ctx[:, kt, :])

                    # whh accumulates into the same psum, k(-pair) outer so
                    # the first pairs start while later context slices are
                    # still reducing on DVE.
                    for kt in range(KT):
                        for mt in range(MT):
                            nc.tensor.matmul(
                                gps[:, mt, :],
                                whh3_d[:, kt, mt * 128:(mt + 1) * 128],
                                atth[:, kt, :],
                                start=False,
                                stop=(kt == KT - 1
                                      and mt in (MT // 2 - 1, MT - 1)))

                    lstm_tail(gps, c_d, h_dec, False,
                              emit_cbf=(t < T_OUT - 1),
                              h8_out=(cat8[:, :, :, SLOT_DEC]
                                      if gate_fp8 else None))
                    nc.vector.tensor_copy(out=dec_hs[:, :, t, :],
                                          in_=h_dec)

                # ------------- ToPose + residual ------------------------
                if "dec" in ablate:
                    return
                ops = pA_pool.tile([P, 2, 512], FP32, tag="pA")
                chunks = [(0, 13), (13, 12)]
                for ci, (t0, tn) in enumerate(chunks):
                    n = tn * B
                    for kt in range(KT):
                        nc.tensor.matmul(
                            ops[:, ci, 0:n],
                            tpT[:, kt * P:(kt + 1) * P],
                            dec_hs[:, kt, t0:t0 + tn, :].rearrange(
                                "p t b -> p (t b)"),
                            start=(kt == 0),
                            stop=(kt == KT - 1 and not has_btp))
                    if has_btp:
                        nc.tensor.matmul(
                            ops[:, ci, 0:n], bias_sb["b_tp"][0:1, :],
                            ones_n[0:1, 0:n], start=False, stop=True)
                    nc.vector.tensor_add(
                        oT_sb[:, t0 * B:t0 * B + n],
                        ops[:, ci, 0:n],
                        residT[:, t0 * B:t0 * B + n])
                nc.sync.dma_start(out=out_d[:, :], in_=oT_sb)

            if loop_iters > 1:
                with tc.For_i(0, loop_iters, 1, name="rep"):
                    body()
            else:
                body()

    return nc


# ------------------------------------------------------------- entry point

_model_cache = {}


def _get_model(key):
    if key not in _model_cache:
        bias_flags, gate_fp8 = key
        _model_cache[key] = build_model(bias_flags, gate_fp8=gate_fp8)
    return _model_cache[key]


def make_in_maps(inputs, gate_fp8=GATE_FP8):
    """Host-side packing: returns per-core input maps."""
    w = _prep_weights(inputs, gate_fp8)
    flags = _bias_flags(w)
    x = np.asarray(inputs["x"], dtype=np.float32)
    z = np.asarray(inputs["z"], dtype=np.float32)
    fr = np.asarray(inputs["for_resid"], dtype=np.float32)

    dmask = np.zeros((B, B, TCAT), dtype=np.float32)
    for b in range(B):
        dmask[b, b, :] = 1.0
    shared = {
        "tfT": w["tfT"], "linT": w["linT"], "tpT": w["tpT"],
        "dmask": np.ascontiguousarray(
            dmask.reshape(B, B * TCAT)).astype(BF16),
    }
    for nm in ("e", "p", "d"):
        shared[f"wih_{nm}"] = w[f"wih_{nm}"]
        shared[f"whh_{nm}"] = w[f"whh_{nm}"]
    names = ("b_tf", "b_e", "b_p", "b_d", "b_lin", "b_tp")
    for f, name in zip(flags, names):
        if f:
            shared[name] = np.ascontiguousarray(
                w[name][None, :]).astype(BF16)

    in_maps = []
    for c in range(N_CORES):
        sl = slice(c * B, (c + 1) * B)
        m = dict(shared)
        m["xT"] = np.ascontiguousarray(
            x[sl].transpose(2, 1, 0).reshape(P, T_IN * B)).astype(BF16)
        m["zT"] = np.ascontiguousarray(
            z[sl].transpose(2, 1, 0).reshape(P, T_IN * B)).astype(BF16)
        m["residT"] = np.ascontiguousarray(
            fr[sl].transpose(2, 1, 0).reshape(P, T_OUT * B))
        in_maps.append(m)
    return in_maps, flags


def unshard_output(results):
    outs = []
    for c in range(N_CORES):
        oT = np.asarray(results[c]["oT"])  # (66, 800)
        outs.append(oT.reshape(P, T_OUT, B).transpose(2, 1, 0))
    return np.ascontiguousarray(np.concatenate(outs, axis=0),
                                dtype=np.float32)


def kernel(**inputs) -> np.ndarray:
    in_maps, flags = make_in_maps(inputs)
    nc = _get_model((flags, GATE_FP8))
    res = run_bass_kernel_spmd(nc, in_maps, core_ids=list(range(N_CORES)))
    return unshard_output(res.results)


# revision 55
# speedup vs baseline: 1.1171x; 1.1101x over previous
"""Trainium2 Bass kernel for nn_AttentionModel (seq2seq LSTM with attention).

Sharding: pure data parallelism over batch (256 -> 8 cores x 32), all
weights replicated. Per-core layout keeps the hidden/gate dimension on
SBUF partitions and (time, batch) on the free axis so the recurrent
matmuls, elementwise gate math, and attention all use one consistent
layout with no on-device transposes.

v2 performance structure (the kernel is LDWEIGHTS-bound: per 128x128
weight tile the PE pays ~128 rows of load for only B=32 moving columns):
- all gate weights (whh_e/p/d, wih_e/p/d, linT) stored fp8 e4m3 and
  consumed with DoubleRow matmuls: each instruction contracts a 256-row
  k-pair, halving both weight-load time and instruction count
- decoder wih+whh gates accumulate into ONE psum tile (no identity-
  matmul merge passes, no psum->sbuf->psum round trips)
- encoder per-step gx add is 2 wide identity matmuls (psum init), not
  32 narrow ones
- the two encoder chains are interleaved so one chain's gate/cell
  elementwise tail (ACT/DVE) hides under the other chain's recurrent
  matmuls (PE)
- gates packed [i, f, o, g] so one Sigmoid covers i,f,o contiguously
- attention scores via a diagonal matmul (c stationary) so the softmax
  lands batch-on-partitions and runs as per-partition-scalar ops

Self-contained: includes the TileContext wait-split workaround and all
host-side packing. The graded entry point is kernel(**inputs).
"""

import numpy as np
import ml_dtypes

import concourse.bass as bass
import concourse.mybir as mybir
import concourse.tile as tile
from concourse.bass_isa import ReduceOp
from concourse.bass_utils import run_bass_kernel_spmd

BF16 = ml_dtypes.bfloat16
FP8 = ml_dtypes.float8_e4m3
FP32 = mybir.dt.float32
BF = mybir.dt.bfloat16
F8 = mybir.dt.float8e4

GATE_FP8 = True     # fp8 gate weights (2x faster LDWEIGHTS + DoubleRow)
DOUBLE_ROW = True   # contract k-tile pairs per matmul instruction

N_CORES = 8
B = 32            # batch per core
T_IN = 10
T_OUT = 25
H = 1024
F = 512
P = 66
G = 4 * H         # 4096 gates
KT = H // 128     # 8  k-tiles over hidden
FT = F // 128     # 4  k-tiles over feature
MT = G // 128     # 32 m-tiles over gates
TCAT = 2 * T_IN + 1   # 21 attention slots
SLOT_DEC = T_IN       # decoder h lives at slot 10

_MAX_WAITS = 1


def _apply_tile_wait_patches():
    """The walrus CoreV3 codegen in this container rejects instructions
    carrying more than one sync-wait command ("Too many sync wait
    commands"). Keep every instruction at <=1 wait by moving excess waits
    onto same-engine nops emitted immediately before the instruction."""
    import bass_rust
    from concourse.vector_clock import ScopedClock

    SyncInfo = bass_rust.SyncInfo

    def _split_waits(nc, inst):
        si = getattr(inst, "sync_info", None)
        if si is None or not si.on_wait or len(si.on_wait) <= _MAX_WAITS:
            return
        if inst.engine == mybir.EngineType.Unassigned:
            return
        waits = list(si.on_wait)
        si.on_wait = waits[:_MAX_WAITS]
        rest = waits[_MAX_WAITS:]
        eng = nc.engines[inst.engine]
        for i in range(0, len(rest), _MAX_WAITS):
            nop = eng.nop(nofuse=True, hint="wait_split")
            nop.ins.sync_info = SyncInfo(
                on_wait=rest[i:i + _MAX_WAITS], on_update=[]
            )

    orig_commit = tile.TileContext._commit_instruction

    def _commit_split(self, inst, lazy_reg_writes=True):
        si = getattr(inst, "sync_info", None)
        if (si is not None and si.on_wait is not None
                and len(si.on_wait) > _MAX_WAITS
                and inst.engine != mybir.EngineType.Unassigned):
            _split_waits(self.nc, inst)
        return orig_commit(self, inst, lazy_reg_writes)

    tile.TileContext._commit_instruction = _commit_split

    def _drain_and_barrier_split(self, tick_clock, wait_clock):
        drain_inst = self.nc.sync.drain()
        wait_clock.add_sem_waits(
            drain_inst.ins, ScopedClock({None: tick_clock.global_clock})
        )
        sync_info = drain_inst.ins.sync_info
        if sync_info is not None and sync_info.on_wait is not None:
            waits = list(sync_info.on_wait)
            if len(waits) > _MAX_WAITS:
                sync_info.on_wait = waits[:_MAX_WAITS]
                rest = waits[_MAX_WAITS:]
                for i in range(0, len(rest), _MAX_WAITS):
                    nop = self.nc.sync.nop(nofuse=True, hint="drain_wait_split")
                    nop.ins.sync_info = SyncInfo(
                        on_wait=rest[i:i + _MAX_WAITS], on_update=[]
                    )
        self.nc.all_engine_barrier()
        assert self.sems is not None
        popped = self.nc._tile_sem_poison_stack.pop()
        assert popped is self._sem_poison
        self.nc.clear_and_free_semaphores(list(self.sems.allocated().values()))
        self.nc.all_engine_barrier()

    tile.TileContext._drain_and_barrier = _drain_and_barrier_split


_apply_tile_wait_patches()

# Opt-in: flip walrus --enable-ldw-opt (hardcoded false in
# bass_utils.bir_verify_and_optimise) by rewriting the command line.
LDW_OPT = False


def _apply_ldw_opt_patch():
    from concourse import bass_utils as _bu
    if getattr(_bu, "_ldw_opt_patched", False):
        return
    _bu._ldw_opt_patched = True
    _orig_run_command = _bu.run_command

    def _run_command_ldw(argv, **kwargs):
        if LDW_OPT:
            argv = ["--enable-ldw-opt=true" if a == "--enable-ldw-opt=false"
                    else a for a in argv]
        return _orig_run_command(argv, **kwargs)

    _bu.run_command = _run_command_ldw


_apply_ldw_opt_patch()


# ------------------------------------------------------------- host packing

# gate reorder: reference packs gates [i, f, g, o]; we use [i, f, o, g]
# so the three sigmoids (i, f, o) are contiguous.
_GPERM = np.concatenate([
    np.arange(0, H),              # i
    np.arange(H, 2 * H),          # f
    np.arange(3 * H, 4 * H),      # o
    np.arange(2 * H, 3 * H),      # g
])


def _pack_T(w, ktiles, mcols, dt=BF16):
    """(mcols, ktiles*128) weight -> transposed tiled layout
    (128, ktiles*mcols) with [p, kt*mcols + m] = w[m, kt*128 + p]."""
    wT = np.ascontiguousarray(np.asarray(w, np.float32).T).astype(dt)
    return np.ascontiguousarray(
        wT.reshape(ktiles, 128, mcols).transpose(1, 0, 2)
        .reshape(128, ktiles * mcols))


WS = 16.0   # fp8 pre-quantization scale: lifts glorot weights and unit
            # activations out of e4m3's subnormal range (the PE flushes
            # subnormal fp8 operands); gates come out x256, undone by the
            # activation's scale argument.


def _prep_weights(inputs, gate_fp8=GATE_FP8):
    gdt = FP8 if gate_fp8 else BF16
    ws = WS if gate_fp8 else 1.0
    d = {}
    d["tfT"] = np.ascontiguousarray(
        np.asarray(inputs["tf_w"], np.float32).T).astype(BF16)  # (66, 512)
    for nm, wih, whh in (("e", "enc_wih", "enc_whh"),
                         ("p", "encp_wih", "encp_whh"),
                         ("d", "dec_wih", "dec_whh")):
        d[f"wih_{nm}"] = _pack_T(
            np.asarray(inputs[wih], np.float32)[_GPERM] * ws, FT, G, gdt)
        d[f"whh_{nm}"] = _pack_T(
            np.asarray(inputs[whh], np.float32)[_GPERM] * ws, KT, G, gdt)
    d["linT"] = _pack_T(
        np.asarray(inputs["lin_w"], np.float32) * ws, KT, F, gdt)
    d["tpT"] = _pack_T(inputs["tp_w"], KT, P)            # (128, 8*66)
    d["b_tf"] = np.asarray(inputs["tf_b"], np.float32)
    for nm, bi, bh in (("e", "enc_bih", "enc_bhh"),
                       ("p", "encp_bih", "encp_bhh"),
                       ("d", "dec_bih", "dec_bhh")):
        d[f"b_{nm}"] = (np.asarray(inputs[bi], np.float32)
                        + np.asarray(inputs[bh], np.float32))[_GPERM] * ws * ws
    d["b_lin"] = np.asarray(inputs["lin_b"], np.float32) * ws * ws
    d["b_tp"] = np.asarray(inputs["tp_b"], np.float32)
    return d


def _bias_flags(w):
    return tuple(bool(np.any(w[k])) for k in
                 ("b_tf", "b_e", "b_p", "b_d", "b_lin", "b_tp"))


# ------------------------------------------------------------ device build

def build_model(bias_flags=(False,) * 6, loop_iters=1, gate_fp8=GATE_FP8,
                double_row=DOUBLE_ROW, ablate=(), warm_fillers=True):
    has_btf, has_be, has_bp, has_bd, has_blin, has_btp = bias_flags
    any_bias = any(bias_flags)
    GDT = F8 if gate_fp8 else BF
    PM = (mybir.MatmulPerfMode.DoubleRow
          if (gate_fp8 and double_row) else None)

    ws = WS if gate_fp8 else 1.0
    gsc = 1.0 / (ws * ws)   # gates psum carries ws^2; undone inside ACT

    nc = bass.Bass()

    xT_d = nc.dram_tensor("xT", [P, T_IN * B], BF, kind="ExternalInput")
    zT_d = nc.dram_tensor("zT", [P, T_IN * B], BF, kind="ExternalInput")
    residT_d = nc.dram_tensor("residT", [P, T_OUT * B], FP32,
                              kind="ExternalInput")
    tfT_d = nc.dram_tensor("tfT", [P, F], BF, kind="ExternalInput")
    wih_d_d = {}
    whh_d_d = {}
    for nm in ("e", "p", "d"):
        wih_d_d[nm] = nc.dram_tensor(f"wih_{nm}", [128, FT * G], GDT,
                                     kind="ExternalInput")
        whh_d_d[nm] = nc.dram_tensor(f"whh_{nm}", [128, KT * G], GDT,
                                     kind="ExternalInput")
    linT_d = nc.dram_tensor("linT", [128, KT * F], GDT, kind="ExternalInput")
    dmask_d = nc.dram_tensor("dmask", [B, B * TCAT], BF, kind="ExternalInput")
    tpT_d = nc.dram_tensor("tpT", [128, KT * P], BF, kind="ExternalInput")
    bias_d = {}
    if has_btf:
        bias_d["b_tf"] = nc.dram_tensor("b_tf", [1, F], BF,
                                        kind="ExternalInput")
    if has_be:
        bias_d["b_e"] = nc.dram_tensor("b_e", [1, G], BF,
                                       kind="ExternalInput")
    if has_bp:
        bias_d["b_p"] = nc.dram_tensor("b_p", [1, G], BF,
                                       kind="ExternalInput")
    if has_bd:
        bias_d["b_d"] = nc.dram_tensor("b_d", [1, G], BF,
                                       kind="ExternalInput")
    if has_blin:
        bias_d["b_lin"] = nc.dram_tensor("b_lin", [1, F], BF,
                                         kind="ExternalInput")
    if has_btp:
        bias_d["b_tp"] = nc.dram_tensor("b_tp", [1, P], BF,
                                        kind="ExternalInput")
    out_d = nc.dram_tensor("oT", [P, T_OUT * B], FP32, kind="ExternalOutput")

    with tile.TileContext(nc) as tc:
        with (
            tc.tile_pool(name="singles", bufs=1) as singles,
            tc.tile_pool(name="wih_pool", bufs=2) as wih_pool,
            tc.tile_pool(name="whh_pool", bufs=2) as whh_pool,
            tc.tile_pool(name="gx_pool", bufs=2) as gx_pool,
            tc.tile_pool(name="ew_pool", bufs=4) as ew_pool,
            tc.tile_pool(name="pA_pool", bufs=1, space="PSUM") as pA_pool,
            tc.tile_pool(name="pB_pool", bufs=1, space="PSUM") as pB_pool,
            tc.tile_pool(name="pC_pool", bufs=1, space="PSUM") as pC_pool,
            tc.tile_pool(name="mps_pool", bufs=2, space="PSUM") as mps_pool,
        ):
            def body(_it=None):
                def cast_scaled(out, in_, s):
                    # fp8 cast with pre-scale (lifts values out of the
                    # subnormal-flush range of the PE's fp8 datapath)
                    if s == 1.0:
                        nc.vector.tensor_copy(out=out, in_=in_)
                    else:
                        nc.vector.tensor_scalar_mul(out, in_, float(s))

                # k_loop: contract kt-tiles of w3 [128, ktn, M] against
                # mov3 [128, ktn, N] into psum `out`; DoubleRow pairs
                # k-tiles when enabled.
                def k_loop(out, w3, mov3, ktn, first, last, pm=None):
                    # pm=DoubleRow only pays off for wide moving operands:
                    # a DR stationary loads at 1 row/cycle unoverlapped
                    # (256 cyc), while consecutive non-DR matmuls hide the
                    # weight load under the previous matmul's moving phase
                    # (~64-74 cyc for K=128). Narrow (B=32) k-loops must
                    # stay non-DR.
                    if pm is not None:
                        for kp in range(ktn // 2):
                            nc.tensor.matmul(
                                out, w3[:, 2 * kp:2 * kp + 2, :],
                                mov3[:, 2 * kp:2 * kp + 2, :],
                                start=(first and kp == 0),
                                stop=(last and kp == ktn // 2 - 1),
                                perf_mode=pm)
                    else:
                        for kt in range(ktn):
                            nc.tensor.matmul(
                                out, w3[:, kt, :], mov3[:, kt, :],
                                start=(first and kt == 0),
                                stop=(last and kt == ktn - 1))

                # ------------- constant/static loads --------------------
                tfT = singles.tile([P, F], BF, tag="tfT")
                nc.sync.dma_start(out=tfT, in_=tfT_d[:, :])
                xT = singles.tile([P, T_IN * B], BF, tag="xT")
                nc.sync.dma_start(out=xT, in_=xT_d[:, :])
                zT = singles.tile([P, T_IN * B], BF, tag="zT")
                nc.sync.dma_start(out=zT, in_=zT_d[:, :])

                wih_e = wih_pool.tile([128, FT * G], GDT, tag="wih")
                for kt in range(FT):
                    nc.sync.dma_start(out=wih_e[:, kt * G:(kt + 1) * G],
                                      in_=wih_d_d["e"][:, kt * G:(kt + 1) * G])
                wih_p = wih_pool.tile([128, FT * G], GDT, tag="wih")
                for kt in range(FT):
                    nc.sync.dma_start(out=wih_p[:, kt * G:(kt + 1) * G],
                                      in_=wih_d_d["p"][:, kt * G:(kt + 1) * G])
                whh_e = whh_pool.tile([128, KT * G], GDT, tag="whh")
                for kt in range(KT):
                    nc.sync.dma_start(out=whh_e[:, kt * G:(kt + 1) * G],
                                      in_=whh_d_d["e"][:, kt * G:(kt + 1) * G])
                whh_p = whh_pool.tile([128, KT * G], GDT, tag="whh")
                for kt in range(KT):
                    nc.sync.dma_start(out=whh_p[:, kt * G:(kt + 1) * G],
                                      in_=whh_d_d["p"][:, kt * G:(kt + 1) * G])

                residT = singles.tile([P, T_OUT * B], FP32, tag="residT")
                nc.sync.dma_start(out=residT, in_=residT_d[:, :])
                linT = singles.tile([128, KT * F], GDT, tag="linT")
                for kt in range(KT):
                    nc.sync.dma_start(out=linT[:, kt * F:(kt + 1) * F],
                                      in_=linT_d[:, kt * F:(kt + 1) * F])
                tpT = singles.tile([128, KT * P], BF, tag="tpT")
                nc.sync.dma_start(out=tpT, in_=tpT_d[:, :])
                dmask = singles.tile([B, B * TCAT], BF, tag="dmask")
                nc.sync.dma_start(out=dmask, in_=dmask_d[:, :])

                bias_sb = {}
                for key, dram in bias_d.items():
                    t = singles.tile(list(dram.shape), BF, tag=key)
                    nc.sync.dma_start(out=t, in_=dram[:, :])
                    bias_sb[key] = t

                ones32 = singles.tile([B, 128], BF, tag="ones32")
                nc.vector.memset(ones32, 1.0)
                ident = singles.tile([128, 128], BF, tag="ident")
                from concourse.masks import make_identity
                make_identity(nc, ident)
                if any_bias:
                    ones_n = singles.tile([1, T_IN * B], BF, tag="ones_n")
                    nc.vector.memset(ones_n, 1.0)

                cat = singles.tile([128, KT, B, TCAT], BF, tag="cat")
                c_e = singles.tile([128, KT, B, 1], FP32, tag="c_e")
                c_p = singles.tile([128, KT, B, 1], FP32, tag="c_p")

                # c_bf doubles as the fp8 scores stationary (ws-scaled)
                c_bf = singles.tile([128, KT, B, 1], GDT, tag="c_bf")
                atth = singles.tile([128, KT, B], GDT, tag="atth")
                inp_bf = singles.tile([128, FT, B], GDT, tag="inp_bf")
                e_bc = singles.tile([128, B * TCAT], BF, tag="e_bc")
                masked32 = singles.tile([B, B * TCAT], FP32, tag="masked32")
                scoresbt = singles.tile([B, TCAT], FP32, tag="scoresbt")
                neg_mx = singles.tile([B, 1], FP32, tag="neg_mx")
                e32 = singles.tile([B, TCAT], FP32, tag="e32")
                ssum32 = singles.tile([B, 1], FP32, tag="ssum32")
                rs32 = singles.tile([B, 1], FP32, tag="rs32")
                attw32 = singles.tile([B, TCAT], BF, tag="attw32")
                aw_m = singles.tile([B, B * TCAT], BF, tag="aw_m")
                prod2 = singles.tile([128, B * TCAT], BF, tag="prod2")
                oT_sb = singles.tile([P, T_OUT * B], FP32, tag="oT_sb")
                # fp8 mirror of cat (ws-scaled): recurrent matmul moving,
                # scores moving, and context source
                cat8 = None
                if gate_fp8:
                    cat8 = singles.tile([128, KT, B, TCAT], F8, tag="cat8")

                XDT = GDT
                xf = singles.tile([128, FT, T_IN * B], XDT, tag="xf")
                zf = singles.tile([128, FT, T_IN * B], XDT, tag="zf")

                wih3_e = wih_e.rearrange("p (kt g) -> p kt g", g=G)
                whh3_e = whh_e.rearrange("p (kt g) -> p kt g", g=G)
                whh3_p = whh_p.rearrange("p (kt g) -> p kt g", g=G)
                lin3 = linT.rearrange("p (kt f) -> p kt f", f=F)

                # ------------- ToFeature --------------------------------
                def to_feature(src, dst):
                    for ft in range(FT):
                        ps = mps_pool.tile([128, T_IN * B], FP32, tag="mps")
                        nc.tensor.matmul(ps, tfT[:, ft * 128:(ft + 1) * 128],
                                         src[:, :], start=True,
                                         stop=not has_btf)
                        if has_btf:
                            nc.tensor.matmul(
                                ps,
                                bias_sb["b_tf"][0:1, ft * 128:(ft + 1) * 128],
                                ones_n[0:1, :], start=False, stop=True)
                        cast_scaled(dst[:, ft, :], ps, ws)

                to_feature(xT, xf)
                to_feature(zT, zf)

                # ------------- encoder gates_x precompute ----------------
                # gx layout: (128, T_IN, MT, B) so per-step slices are
                # contiguous.
                def gates_x(wih3, src, dst, bkey):
                    for mt in range(MT):
                        ps = mps_pool.tile([128, T_IN * B], FP32, tag="mps")
                        k_loop(ps, wih3[:, :, mt * 128:(mt + 1) * 128],
                               src, FT, True, bkey is None, pm=PM)
                        if bkey is not None:
                            nc.tensor.matmul(
                                ps, bias_sb[bkey][0:1, mt * 128:(mt + 1) * 128],
                                ones_n[0:1, :], start=False, stop=True)
                        nc.vector.tensor_copy(
                            out=dst[:, :, mt, :],
                            in_=ps.rearrange("p (t b) -> p t b", b=B))

                gx_e = gx_pool.tile([128, T_IN, MT, B], BF, tag="gx")
                gates_x(wih3_e, xf, gx_e, "b_e" if has_be else None)

                wih3_p = wih_p.rearrange("p (kt g) -> p kt g", g=G)
                gx_p = gx_pool.tile([128, T_IN, MT, B], BF, tag="gx")
                gates_x(wih3_p, zf, gx_p, "b_p" if has_bp else None)

                wih_dd = wih_pool.tile([128, FT * G], GDT, tag="wih")
                for kt in range(FT):
                    nc.sync.dma_start(out=wih_dd[:, kt * G:(kt + 1) * G],
                                      in_=wih_d_d["d"][:, kt * G:(kt + 1) * G])
                wih3_d = wih_dd.rearrange("p (kt g) -> p kt g", g=G)

                def pe_filler(dep_ap):
                    # Tiny matmul dependent on a just-produced DVE/ACT tile.
                    # Keeps the PE HAM activity window busy during long
                    # DVE/ACT chains so the next real matmul block starts at
                    # 2.4 GHz instead of re-warming from 1.2 GHz.
                    if not warm_fillers:
                        return
                    fps = mps_pool.tile([1, 8], FP32, tag="mps")
                    nc.tensor.matmul(fps[:, 0:1], dep_ap, dep_ap,
                                     start=True, stop=True)

                # ------------- LSTM gate elementwise ---------------------
                # gates packed [i, f, o, g]: one Sigmoid over [0:3H], one
                # Tanh over g; ACT never does copies.
                def lstm_tail(gsrc, c_tile, h_out, first_step,
                              emit_cbf=False, h8_out=None):
                    sio = ew_pool.tile([128, 3 * KT, B], FP32, tag="ew")
                    tg = ew_pool.tile([128, KT, B], FP32, tag="ew")
                    # sigmoid(x) = 0.5*tanh(x/2) + 0.5: keeps every ACT op
                    # in the tail on the Tanh LUT (a function switch reloads
                    # the LUT, ~1.3us), at the cost of one wide DVE affine
                    nc.scalar.activation(
                        out=sio, in_=gsrc[:, 0:3 * KT, :],
                        func=mybir.ActivationFunctionType.Tanh,
                        scale=gsc * 0.5)
                    nc.vector.tensor_scalar(
                        sio, sio, 0.5, 0.5,
                        op0=mybir.AluOpType.mult, op1=mybir.AluOpType.add)
                    nc.scalar.activation(
                        out=tg, in_=gsrc[:, 3 * KT:4 * KT, :],
                        func=mybir.ActivationFunctionType.Tanh,
                        scale=gsc)
                    si = sio[:, 0:KT, :]
                    sf = sio[:, KT:2 * KT, :]
                    so = sio[:, 2 * KT:3 * KT, :]
                    cs = c_tile[:, :, :, 0]
                    pe_filler(sio[:, 0:1, 0:1])
                    # tg <- i*g
                    nc.vector.tensor_mul(tg, si, tg)
                    if first_step:
                        nc.vector.tensor_copy(out=cs, in_=tg)
                    else:
                        nc.vector.tensor_mul(cs, sf, cs)
                        nc.vector.tensor_add(cs, cs, tg)
                    th = ew_pool.tile([128, KT, B], FP32, tag="ew")
                    nc.scalar.activation(
                        out=th, in_=cs,
                        func=mybir.ActivationFunctionType.Tanh)
                    if emit_cbf:
                        # next decoder step's attention stationary, cast
                        # while ACT computes tanh / h
                        cast_scaled(c_bf, c_tile, ws)
                    nc.vector.tensor_mul(h_out, so, th)
                    if h8_out is not None:
                        cast_scaled(h8_out, h_out, ws)

                # ------------- encoder chains (interleaved) --------------
                def enc_step(whh3, gx, c_tile, slot, t, gpool):
                    h_out = cat[:, :, :, slot]
                    h8 = cat8[:, :, :, slot] if gate_fp8 else None
                    if t == 0:
                        lstm_tail(gx[:, 0, :, :], c_tile, h_out, True,
                                  h8_out=h8)
                        return
                    gps = gpool.tile([128, MT, B], FP32, tag="g")
                    # init psum with gx[t] via two wide identity matmuls.
                    # PSUM start_tensor_calc marks a whole 2KB bank pending-
                    # zero, so exactly ONE matmul per bank may carry
                    # start=True; every other matmul accumulates (first
                    # touch of a pending byte auto-replaces).
                    half = MT // 2
                    for ci in range(2):
                        nc.tensor.matmul(
                            gps[:, ci * half:(ci + 1) * half, :], ident,
                            gx[:, t, ci * half:(ci + 1) * half, :],
                            start=True, stop=False)
                    prev = (cat8 if gate_fp8 else cat)[:, :, :, slot - 1]
                    for mt in range(MT):
                        k_loop(gps[:, mt, :],
                               whh3[:, :, mt * 128:(mt + 1) * 128],
                               prev, KT, False,
                               mt in (MT // 2 - 1, MT - 1))
                    lstm_tail(gps, c_tile, h_out, False, h8_out=h8)

                if "enc" not in ablate:
                    for t in range(T_IN):
                        enc_step(whh3_e, gx_e, c_e, t, t, pB_pool)
                        enc_step(whh3_p, gx_p, c_p, T_IN + 1 + t, t, pC_pool)
                else:
                    nc.vector.memset(cat, 0.01)
                    nc.vector.memset(c_e, 0.01)
                    nc.vector.memset(c_p, 0.01)
                    if gate_fp8:
                        nc.vector.memset(cat8, 0.01)

                # ------------- decoder ----------------------------------
                whh_dd = whh_pool.tile([128, KT * G], GDT, tag="whh")
                for kt in range(KT):
                    nc.sync.dma_start(out=whh_dd[:, kt * G:(kt + 1) * G],
                                      in_=whh_d_d["d"][:, kt * G:(kt + 1) * G])
                whh3_d = whh_dd.rearrange("p (kt g) -> p kt g", g=G)

                dec_hs = gx_pool.tile([128, KT, T_OUT, B], BF, tag="gx")

                nc.vector.tensor_copy(out=cat[:, :, :, SLOT_DEC],
                                      in_=cat[:, :, :, T_IN - 1])
                c_d = c_e
                cast_scaled(c_bf, c_d, ws)
                if gate_fp8:
                    nc.vector.tensor_copy(out=cat8[:, :, :, SLOT_DEC],
                                          in_=cat8[:, :, :, T_IN - 1])

                nhalf = B // 2
                h_dec = cat[:, :, :, SLOT_DEC]
                h_mov = (cat8 if gate_fp8 else cat)[:, :, :, SLOT_DEC]

                dec_steps = 0 if "dec" in ablate else T_OUT
                for t in range(dec_steps):
                    # lin: inp = h @ lin_w.T
                    ips = mps_pool.tile([128, FT, B], FP32, tag="mps")
                    for mt in range(FT):
                        k_loop(ips[:, mt, :],
                               lin3[:, :, mt * 128:(mt + 1) * 128],
                               h_mov, KT, True, not has_blin)
                        if has_blin:
                            nc.tensor.matmul(
                                ips[:, mt, :],
                                bias_sb["b_lin"][0:1, mt * 128:(mt + 1) * 128],
                                ones_n[0:1, 0:B], start=False, stop=True)
                    cast_scaled(inp_bf, ips, 1.0 / ws)

                    # gates psum: wih part first (runs under the softmax
                    # chain), whh part accumulates directly afterwards.
                    gps = (pB_pool if t % 2 == 0 else pC_pool).tile(
                        [128, MT, B], FP32, tag="g")

                    def emit_gpw(mts=range(MT)):
                        # one start=True per 2KB psum bank (mt 0 and 16);
                        # all other writes rely on pending-zero auto-replace
                        for mt in mts:
                            k_loop(gps[:, mt, :],
                                   wih3_d[:, :, mt * 128:(mt + 1) * 128],
                                   inp_bf, FT, mt in (0, MT // 2), False)
                            if has_bd:
                                nc.tensor.matmul(
                                    gps[:, mt, :],
                                    bias_sb["b_d"][0:1,
                                                   mt * 128:(mt + 1) * 128],
                                    ones_n[0:1, 0:B], start=False, stop=False)

                    if t == 0 and "attn" in ablate:
                        nc.vector.memset(atth, 0.01)
                    if "attn" in ablate:
                        emit_gpw()
                    if "attn" not in ablate:
                        # scores via diagonal matmul: out[b,(b',t)] =
                        # sum_h c[h,b] cat[h,b',t]; diagonal extracted by
                        # mask-multiply + strided reduce. Scores land
                        # batch-on-partitions -> softmax is 4 cheap ops.
                        scd = pA_pool.tile([B, 2, 512], FP32, tag="pA")
                        if PM is not None:
                            for kp in range(KT // 2):
                                sl = slice(2 * kp, 2 * kp + 2)
                                nc.tensor.matmul(
                                    scd[:, 0, 0:nhalf * TCAT],
                                    c_bf[:, sl, :, 0],
                                    cat8[:, sl, 0:nhalf, :],
                                    start=(kp == 0), stop=(kp == KT // 2 - 1),
                                    perf_mode=PM)
                                nc.tensor.matmul(
                                    scd[:, 1, 0:nhalf * TCAT],
                                    c_bf[:, sl, :, 0],
                                    cat8[:, sl, nhalf:B, :],
                                    start=(kp == 0), stop=(kp == KT // 2 - 1),
                                    perf_mode=PM)
                        else:
                            sc_mov = cat8 if gate_fp8 else cat
                            for kt in range(KT):
                                nc.tensor.matmul(
                                    scd[:, 0, 0:nhalf * TCAT],
                                    c_bf[:, kt, :, 0],
                                    sc_mov[:, kt, 0:nhalf, :],
                                    start=(kt == 0), stop=(kt == KT - 1))
                                nc.tensor.matmul(
                                    scd[:, 1, 0:nhalf * TCAT],
                                    c_bf[:, kt, :, 0],
                                    sc_mov[:, kt, nhalf:B, :],
                                    start=(kt == 0), stop=(kt == KT - 1))
                        # first wih half only: eall (which gates the whole
                        # context chain) must not queue behind all 128 wih
                        # matmuls on the in-order PE
                        emit_gpw(range(0, MT // 2))
                        nc.vector.tensor_mul(
                            masked32.rearrange("p (c n) -> p c n", c=2),
                            scd[:, :, 0:nhalf * TCAT],
                            dmask.rearrange("p (c n) -> p c n", c=2))
                        nc.vector.tensor_reduce(
                            scoresbt,
                            masked32.rearrange("p (b t) -> p t b", t=TCAT),
                            axis=mybir.AxisListType.X, op=mybir.AluOpType.add)
                        pe_filler(scoresbt[:, 0:1])
                        nc.vector.tensor_reduce(
                            neg_mx, scoresbt, axis=mybir.AxisListType.X,
                            op=mybir.AluOpType.max, negate=True)
                        if gsc != 1.0:
                            # scores psum carries ws^2; exp descales via its
                            # scale arg, the (post-scale) bias needs the same
                            nc.vector.tensor_scalar_mul(neg_mx, neg_mx, gsc)
                        nc.scalar.activation(
                            out=e32, in_=scoresbt,
                            func=mybir.ActivationFunctionType.Exp,
                            bias=neg_mx, scale=gsc, accum_out=ssum32)
                        pe_filler(e32[:, 0:1])
                        nc.vector.reciprocal(rs32, ssum32)
                        nc.vector.tensor_scalar_mul(attw32, e32, rs32)
                        # broadcast attw to all partitions without a DMA:
                        # aw_m[b, (b', t)] = attw[b, t] * dmask, then a K=32
                        # ones matmul sums over b leaving attw[b', t]
                        # replicated on every partition.
                        # broadcast attw to all partitions without a DMA:
                        # aw_m[b, (b', t)] = attw[b, t] * dmask, then a K=32
                        # ones matmul sums over b leaving attw[b', t]
                        # replicated on every partition.
                        nc.vector.tensor_mul(
                            aw_m, dmask,
                            attw32.unsqueeze(1).to_broadcast((B, B, TCAT)))
                        eall = pA_pool.tile([128, 2, 512], FP32, tag="pA")
                        nc.tensor.matmul(eall[:, 0, 0:nhalf * TCAT],
                                         ones32, aw_m[:, 0:nhalf * TCAT],
                                         start=True, stop=True)
                        nc.tensor.matmul(eall[:, 1, 0:nhalf * TCAT],
                                         ones32, aw_m[:, nhalf * TCAT:],
                                         start=True, stop=True)
                        nc.vector.tensor_copy(out=e_bc[:, 0:nhalf * TCAT],
                                              in_=eall[:, 0, 0:nhalf * TCAT])
                        nc.vector.tensor_copy(out=e_bc[:, nhalf * TCAT:],
                                              in_=eall[:, 1, 0:nhalf * TCAT])
                        emit_gpw(range(MT // 2, MT))

                        # context: atth[:, kt, :] = sum_t cat[:, kt] * attw
                        # (from cat8 in fp8 mode, so ctx lands ws-scaled and
                        # atth is a plain copy)
                        ctx_src = cat8 if gate_fp8 else cat
                        ctx = ew_pool.tile([128, KT, B], FP32, tag="ew")
                        for kt in range(KT):
                            nc.vector.tensor_mul(
                                prod2, ctx_src[:, kt, :, :],
                                e_bc.rearrange("p (b t) -> p b t", t=TCAT))
                            nc.vector.tensor_reduce(
                                ctx[:, kt, :],
                                prod2.rearrange("p (b t) -> p b t", t=TCAT),
                                axis=mybir.AxisListType.X,
                                op=mybir.AluOpType.add)
                            nc.vector.tensor_copy(out=atth[:, kt, :],
                                                  in_=ctx[:, kt, :])

                    # whh accumulates into the same psum, k(-pair) outer so
                    # the first pairs start while later context slices are
                    # still reducing on DVE.
                    for kt in range(KT):
                        for mt in range(MT):
                            nc.tensor.matmul(
                                gps[:, mt, :],
                                whh3_d[:, kt, mt * 128:(mt + 1) * 128],
                                atth[:, kt, :],
                                start=False,
                                stop=(kt == KT - 1
                                      and mt in (MT // 2 - 1, MT - 1)))

                    lstm_tail(gps, c_d, h_dec, False,
                              emit_cbf=(t < T_OUT - 1),
                              h8_out=(cat8[:, :, :, SLOT_DEC]
                                      if gate_fp8 else None))
                    nc.vector.tensor_copy(out=dec_hs[:, :, t, :],
                                          in_=h_dec)

                # ------------- ToPose + residual ------------------------
                if "dec" in ablate:
                    return
                ops = pA_pool.tile([P, 2, 512], FP32, tag="pA")
                chunks = [(0, 13), (13, 12)]
                for ci, (t0, tn) in enumerate(chunks):
                    n = tn * B
                    for kt in range(KT):
                        nc.tensor.matmul(
                            ops[:, ci, 0:n],
                            tpT[:, kt * P:(kt + 1) * P],
                            dec_hs[:, kt, t0:t0 + tn, :].rearrange(
                                "p t b -> p (t b)"),
                            start=(kt == 0),
                            stop=(kt == KT - 1 and not has_btp))
                    if has_btp:
                        nc.tensor.matmul(
                            ops[:, ci, 0:n], bias_sb["b_tp"][0:1, :],
                            ones_n[0:1, 0:n], start=False, stop=True)
                    nc.vector.tensor_add(
                        oT_sb[:, t0 * B:t0 * B + n],
                        ops[:, ci, 0:n],
                        residT[:, t0 * B:t0 * B + n])
                nc.sync.dma_start(out=out_d[:, :], in_=oT_sb)

            if loop_iters > 1:
                with tc.For_i(0, loop_iters, 1, name="rep"):
                    body()
            else:
                body()

    return nc


# ------------------------------------------------------------- entry point

_model_cache = {}


def _get_model(key):
    if key not in _model_cache:
        bias_flags, gate_fp8 = key
        _model_cache[key] = build_model(bias_flags, gate_fp8=gate_fp8)
    return _model_cache[key]


def make_in_maps(inputs, gate_fp8=GATE_FP8):
    """Host-side packing: returns per-core input maps."""
    w = _prep_weights(inputs, gate_fp8)
    flags = _bias_flags(w)
    x = np.asarray(inputs["x"], dtype=np.float32)
    z = np.asarray(inputs["z"], dtype=np.float32)
    fr = np.asarray(inputs["for_resid"], dtype=np.float32)

    dmask = np.zeros((B, B, TCAT), dtype=np.float32)
    for b in range(B):
        dmask[b, b, :] = 1.0
    shared = {
        "tfT": w["tfT"], "linT": w["linT"], "tpT": w["tpT"],
        "dmask": np.ascontiguousarray(
            dmask.reshape(B, B * TCAT)).astype(BF16),
    }
    for nm in ("e", "p", "d"):
        shared[f"wih_{nm}"] = w[f"wih_{nm}"]
        shared[f"whh_{nm}"] = w[f"whh_{nm}"]
    names = ("b_tf", "b_e", "b_p", "b_d", "b_lin", "b_tp")
    for f, name in zip(flags, names):
        if f:
            shared[name] = np.ascontiguousarray(
                w[name][None, :]).astype(BF16)

    in_maps = []
    for c in range(N_CORES):
        sl = slice(c * B, (c + 1) * B)
        m = dict(shared)
        m["xT"] = np.ascontiguousarray(
            x[sl].transpose(2, 1, 0).reshape(P, T_IN * B)).astype(BF16)
        m["zT"] = np.ascontiguousarray(
            z[sl].transpose(2, 1, 0).reshape(P, T_IN * B)).astype(BF16)
        m["residT"] = np.ascontiguousarray(
            fr[sl].transpose(2, 1, 0).reshape(P, T_OUT * B))
        in_maps.append(m)
    return in_maps, flags


def unshard_output(results):
    outs = []
    for c in range(N_CORES):
        oT = np.asarray(results[c]["oT"])  # (66, 800)
        outs.append(oT.reshape(P, T_OUT, B).transpose(2, 1, 0))
    return np.ascontiguousarray(np.concatenate(outs, axis=0),
                                dtype=np.float32)


def kernel(**inputs) -> np.ndarray:
    in_maps, flags = make_in_maps(inputs)
    nc = _get_model((flags, GATE_FP8))
    res = run_bass_kernel_spmd(nc, in_maps, core_ids=list(range(N_CORES)))
    return unshard_output(res.results)
